# revision 1
# baseline (speedup 1.0000x reference)
"""Bass/Tile TRN2 kernel for a 4-layer dense transformer (D=768, H=12, DF=3072,
V=32000, B=2, T=2048) sharded across 8 NeuronCores.

Sharding: each core owns 512 tokens (core c -> batch c//4, tokens 512*(c%4)...)
for the transformer body; K/V are exchanged per layer with an AllGather inside
each 4-core batch group.  For the tied LM head the final hidden states are
AllGathered across all 8 cores and the vocabulary is sharded 4000 (padded 4096)
per core.  The program is identical on every core (SPMD); all causal structure
lives in per-core mask input data.

Layout: activations are kept feature-major in SBUF ([128, 6, 512] = d-major x
tokens), which makes every projection a natural lhsT=W, rhs=x matmul.  The
layernorms are folded into the projections: y = LN(x) @ W is computed as
rstd*(x @ W) with a rank-1 (-mu * colsum(W)) correction matmul, so no
normalized copy of x is ever materialized (valid because this model's LN
scale/bias are identity, asserted on host).
"""

import os
import sys
import time

for _p in ("/opt/trn_rl_repo", "/root/.axon_site/_ro/trn_rl_repo"):
    if os.path.isdir(_p) and _p not in sys.path:
        sys.path.insert(0, _p)

import numpy as np
import ml_dtypes

D, DF, H, L, V, T_MAX = 768, 3072, 12, 4, 32000, 2048
HD = D // H          # 64
B, T = 2, 2048
NCORES = 8
TOK = 512            # tokens per core
DC = D // 128        # 6 feature chunks
DFC = DF // 128      # 24
VSH = V // NCORES    # 4000 vocab per core
VPAD = 4096          # padded vocab shard
NKT = (B * T // NCORES) // 128 * 4  # 16 key chunks of 128 (full 2048 per batch)
EPS = 1e-5

_STATE = {}
ABLATE = os.environ.get("KERNEL_ABLATE", "")


def _build_program():
    import concourse.bass as bass
    import concourse.tile as tile
    from concourse import bacc, mybir
    from concourse.masks import make_identity

    f32 = mybir.dt.float32
    bf16 = mybir.dt.bfloat16
    i32 = mybir.dt.int32
    EXP = mybir.ActivationFunctionType.Exp
    SILU = mybir.ActivationFunctionType.Silu
    SQRT = mybir.ActivationFunctionType.Sqrt

    nc = bacc.Bacc("TRN2", target_bir_lowering=False, debug=False,
                   num_devices=NCORES)

    # ---------------- DRAM I/O ----------------
    # shared (same array on all cores)
    te_d = nc.dram_tensor("te", [V, D], f32, kind="ExternalInput")
    wqkv_d = nc.dram_tensor("wqkv", [L, D, 3 * D], bf16, kind="ExternalInput")
    wout_d = nc.dram_tensor("wout", [L, D, D], bf16, kind="ExternalInput")
    wup_d = nc.dram_tensor("wup", [L, D, DF], bf16, kind="ExternalInput")
    wdn_d = nc.dram_tensor("wdn", [L, DF, D], bf16, kind="ExternalInput")
    # per-core
    idx_d = nc.dram_tensor("idx", [TOK, 1], i32, kind="ExternalInput")
    pe_d = nc.dram_tensor("pe_s", [TOK, D], f32, kind="ExternalInput")
    mask_d = nc.dram_tensor("masks", [NKT, 128, TOK], bf16, kind="ExternalInput")
    teT_d = nc.dram_tensor("teT_s", [D, VPAD], bf16, kind="ExternalInput")
    # output
    out_d = nc.dram_tensor("logits", [NCORES * TOK, VPAD], f32,
                           kind="ExternalOutput")

    # internal DRAM for collectives
    KSZ = D * TOK                 # 393216 elems
    VSZ = TOK * H * (HD + 1)      # 399360 elems
    NKV = KSZ + VSZ
    kv_in = nc.dram_tensor("kv_in", [1, NKV], bf16)
    kv_out = nc.dram_tensor("kv_out", [4, NKV], bf16)
    xh_in = nc.dram_tensor("xh_in", [D + 1, TOK], bf16)
    xh_out = nc.dram_tensor("xh_out", [NCORES * (D + 1), TOK], bf16,
                            addr_space="Shared")

    kvK_in = kv_in[0, 0:KSZ].rearrange("(c p f) -> p c f", c=DC, p=128)
    kvV_in = kv_in[0, KSZ:NKV].rearrange("(tc p h w) -> p tc h w",
                                         tc=4, p=128, h=H)

    def kvK_out(r, hp):
        # [128, 512] slice of rank r's K block: feature rows 128*hp..
        return kv_out[r, 0:KSZ].rearrange("(c p f) -> c p f", c=DC, p=128)[hp]

    def kvV_out(r, tc4):
        return kv_out[r, KSZ:NKV].rearrange("(tc p h w) -> tc p h w",
                                            tc=4, p=128, h=H)[tc4]

    GROUPS4 = [[0, 1, 2, 3], [4, 5, 6, 7]]
    GROUPS8 = [list(range(NCORES))]

    with tile.TileContext(nc) as tc:
        import contextlib
        with contextlib.ExitStack() as ctx:
            # ---------------- pools ----------------
            const = ctx.enter_context(tc.tile_pool(name="const", bufs=1))
            xp = ctx.enter_context(tc.tile_pool(name="xp", bufs=1))
            act = ctx.enter_context(tc.tile_pool(name="act", bufs=1))
            wstream = ctx.enter_context(tc.tile_pool(name="wstream", bufs=6))
            rows = ctx.enter_context(tc.tile_pool(name="rows", bufs=1))
            tmp = ctx.enter_context(tc.tile_pool(name="tmp", bufs=2))
            pbuf = ctx.enter_context(tc.tile_pool(name="pbuf", bufs=4))
            kkp = ctx.enter_context(tc.tile_pool(name="kkp", bufs=2))
            ps_big = ctx.enter_context(
                tc.tile_pool(name="ps_big", bufs=2, space="PSUM"))
            ps_att = ctx.enter_context(
                tc.tile_pool(name="ps_att", bufs=2, space="PSUM"))
            ps_row = ps_att

            # ---------------- constants ----------------
            ones_col = const.tile([128, 1], f32, tag="ones_col")
            nc.vector.memset(ones_col[:], 1.0)
            ones_col_b = const.tile([128, 1], bf16, tag="ones_col_b")
            nc.vector.memset(ones_col_b[:], 1.0)
            ones_row = const.tile([1, 128], f32, tag="ones_row")
            nc.vector.memset(ones_row[:], 1.0)
            eps_t = const.tile([1, 1], f32, tag="eps")
            nc.vector.memset(eps_t[:], EPS)
            ident = const.tile([128, 128], f32, tag="ident")
            make_identity(nc, ident[:])
            masks_sb = const.tile([128, NKT, TOK], bf16, tag="masks")
            nc.sync.dma_start(masks_sb[:], mask_d[:].rearrange("k p f -> p k f"))

            # persistent activations
            x_fm = xp.tile([128, DC, TOK], f32, tag="x_fm")

            # ---------------- embedding ----------------
            idx_sb = tmp.tile([128, 4, 1], i32, tag="idx")
            nc.sync.dma_start(
                idx_sb[:], idx_d[:].rearrange("(tc p) o -> p tc o", p=128))
            for tc4 in range(4):
                emb_t = tmp.tile([128, D], f32, tag="emb")
                nc.gpsimd.indirect_dma_start(
                    out=emb_t[:], out_offset=None, in_=te_d[:],
                    in_offset=bass.IndirectOffsetOnAxis(
                        ap=idx_sb[:, tc4, 0:1], axis=0))
                pe_t = tmp.tile([128, D], f32, tag="pe")
                nc.sync.dma_start(pe_t[:], pe_d[128 * tc4:128 * (tc4 + 1), :])
                nc.vector.tensor_add(emb_t[:], emb_t[:], pe_t[:])
                for dc in range(DC):
                    tp = ps_att.tile([128, 128], f32, tag="att")
                    nc.tensor.transpose(
                        tp[:], emb_t[:, 128 * dc:128 * (dc + 1)], ident[:])
                    nc.vector.tensor_copy(
                        x_fm[:, dc, 128 * tc4:128 * (tc4 + 1)], tp[:])

            # ---------------- helpers ----------------
            def ln_stats():
                """LN statistics of x_fm.  Returns (m2b_row bf16 [1,TOK],
                rstd_bc f32 [128,TOK] sbuf, rstd_row f32 [1,TOK],
                mu_row f32 [1,TOK])."""
                sum_ps = ps_row.tile([1, TOK], f32, tag="att")
                sq_ps = ps_row.tile([1, TOK], f32, tag="att")
                for dc in range(DC):
                    nc.tensor.matmul(sum_ps[:], ones_col[:], x_fm[:, dc, :],
                                     start=(dc == 0), stop=(dc == DC - 1))
                for dc in range(DC):
                    xsq = tmp.tile([128, TOK], f32, tag="xsq")
                    nc.vector.tensor_mul(xsq[:], x_fm[:, dc, :], x_fm[:, dc, :])
                    nc.tensor.matmul(sq_ps[:], ones_col[:], xsq[:],
                                     start=(dc == 0), stop=(dc == DC - 1))
                mu_row = rows.tile([1, TOK], f32, tag="mu")
                nc.vector.tensor_scalar_mul(mu_row[:], sum_ps[:], 1.0 / D)
                ex2 = rows.tile([1, TOK], f32, tag="ex2")
                nc.vector.tensor_scalar_mul(ex2[:], sq_ps[:], 1.0 / D)
                var = rows.tile([1, TOK], f32, tag="var")
                nc.vector.tensor_mul(var[:], mu_row[:], mu_row[:])
                nc.vector.tensor_sub(var[:], ex2[:], var[:])
                std = rows.tile([1, TOK], f32, tag="std")
                nc.scalar.activation(std[:], var[:], SQRT, bias=eps_t[:])
                rstd_row = rows.tile([1, TOK], f32, tag="rstd")
                nc.vector.reciprocal(rstd_row[:], std[:])
                m2b_row = rows.tile([1, TOK], bf16, tag="m2b")
                nc.vector.tensor_scalar_mul(m2b_row[:], mu_row[:], -1.0)
                bc_ps = ps_big.tile([128, TOK], f32, tag="big")
                nc.tensor.matmul(bc_ps[:], ones_row[:], rstd_row[:],
                                 start=True, stop=True)
                rstd_bc = rows.tile([128, TOK], f32, tag="rstd_bc")
                nc.vector.tensor_copy(rstd_bc[:], bc_ps[:])
                return m2b_row, rstd_bc, rstd_row, mu_row

            def cast_x():
                xb = act.tile([128, DC, TOK], bf16, tag="xb")
                for dc in range(DC):
                    nc.vector.tensor_copy(xb[:, dc, :], x_fm[:, dc, :])
                return xb

            def wcol_chunk(src_ap, tag="wchunk", n=128, bufs=None):
                """Stream a [D, n] weight column block into SBUF as
                [128, DC, n] plus its bf16 colsum row [1, n]."""
                wc = wstream.tile([128, DC, n], bf16, tag=tag,
                                  name=f"wc{_uid[0]}", bufs=bufs)
                _uid[0] += 1
                nc.sync.dma_start(
                    wc[:], src_ap.rearrange("(c p) n -> p c n", p=128))
                cps = ps_row.tile([1, n], f32, tag="att", name=f"cps{_uid[0]}")
                for dc in range(DC):
                    nc.tensor.matmul(cps[:], ones_col_b[:], wc[:, dc, :],
                                     start=(dc == 0), stop=(dc == DC - 1))
                cs = rows.tile([1, n], bf16, tag="cs", name=f"cs{_uid[0]}",
                               bufs=2)
                nc.vector.tensor_copy(cs[:], cps[:])
                return wc, cs

            _uid = [0]

            # ---------------- layers ----------------
            for l in range(L):
                # ---- LN1 stats + cast
                m2b, rstd_bc, rstd_row, _mu = ln_stats()
                xb = cast_x()
                # rstd as columns for the V (token-major) projection
                rstd_cols = rows.tile([128, 4], f32, tag="rstd_cols")
                for tc4 in range(4):
                    trp = ps_att.tile([128, 1], f32, tag="att")
                    nc.tensor.transpose(
                        trp[:], rstd_row[:, 128 * tc4:128 * (tc4 + 1)],
                        ident[:1, :1])
                    nc.vector.tensor_copy(rstd_cols[:, tc4:tc4 + 1], trp[:])

                # ---- K projection first (feature-major out), paired chunks
                q_sb = act.tile([128, DC, TOK], bf16, tag="q")
                k_sb = act.tile([128, DC, TOK], bf16, tag="k")

                def qk_proj(dst, base):
                    for ocp in range(3):
                        col0 = base + 256 * ocp
                        wc, cs = wcol_chunk(wqkv_d[l][:, col0:col0 + 256],
                                            n=256)
                        for k2 in range(2):
                            sl = slice(128 * k2, 128 * (k2 + 1))
                            pp = ps_big.tile([128, TOK], f32, tag="big")
                            for dc in range(DC):
                                nc.tensor.matmul(pp[:], wc[:, dc, sl],
                                                 xb[:, dc, :],
                                                 start=(dc == 0), stop=False)
                            nc.tensor.matmul(pp[:], cs[:, sl], m2b[:],
                                             start=False, stop=True)
                            nc.vector.tensor_mul(dst[:, 2 * ocp + k2, :],
                                                 pp[:], rstd_bc[:])

                qk_proj(k_sb, D)

                # ---- V projection (token-major out, with ones column)
                v_loc = act.tile([128, 4, H, HD + 1], bf16, tag="v_loc")
                nc.vector.memset(v_loc[:, :, :, HD:HD + 1], 1.0)
                for nv in range(2):  # 2 chunks of 384 = 6 heads
                    col0 = 2 * D + 384 * nv
                    wv, cv = wcol_chunk(wqkv_d[l][:, col0:col0 + 384],
                                        tag="wv", n=384, bufs=2)
                    for tc4 in range(4):
                        pp = ps_big.tile([128, 384], f32, tag="big")
                        for dc in range(DC):
                            nc.tensor.matmul(
                                pp[:], xb[:, dc, 128 * tc4:128 * (tc4 + 1)],
                                wv[:, dc, :], start=(dc == 0), stop=False)
                        nc.tensor.matmul(
                            pp[:], m2b[:, 128 * tc4:128 * (tc4 + 1)], cv[:],
                            start=False, stop=True)
                        nc.vector.tensor_scalar_mul(
                            v_loc[:, tc4, 6 * nv:6 * (nv + 1), 0:HD],
                            pp[:].rearrange("p (h w) -> p h w", h=6),
                            rstd_cols[:, tc4:tc4 + 1])

                # ---- ship K/V, AllGather within the batch group; Q overlaps
                nc.sync.dma_start(kvK_in, k_sb[:])
                nc.sync.dma_start(kvV_in, v_loc[:])
                nc.gpsimd.collective_compute(
                    "AllGather", mybir.AluOpType.bypass,
                    replica_groups=GROUPS4, ins=[kv_in[:]], outs=[kv_out[:]])

                qk_proj(q_sb, 0)

                # ---- load gathered V
                vv = act.tile([128, NKT, H, HD + 1], bf16, tag="vv")
                for r in range(4):
                    for tc4 in range(4):
                        nc.sync.dma_start(vv[:, 4 * r + tc4, :, :],
                                          kvV_out(r, tc4))

                # ---- attention
                o_sb = act.tile([128, DC, TOK], bf16, tag="o")
                if ABLATE == "attn":
                    nc.vector.memset(o_sb[:], 0.001)
                for hp in range(0 if ABLATE == "attn" else DC):  # head pairs
                    kk = kkp.tile([128, 4, TOK], bf16, tag="kk")
                    for r in range(4):
                        nc.sync.dma_start(kk[:, r, :], kvK_out(r, hp))
                    # both heads of the pair per kt chunk: the two score
                    # matmuls sit at base partitions 0/64 (distinct row
                    # groups) so the PE runs them concurrently
                    o_psA = ps_att.tile([HD + 1, TOK], f32, tag="att",
                                        name=f"opsA{l}_{hp}")
                    o_psB = ps_att.tile([HD + 1, TOK], f32, tag="att",
                                        name=f"opsB{l}_{hp}")
                    o_pss = [o_psA, o_psB]
                    for kt in range(NKT):
                        s2 = ps_big.tile([128, 2, TOK], f32, tag="s2")
                        for h01 in range(2):
                            nc.tensor.matmul(
                                s2[:, h01, :],
                                kk[64 * h01:64 * h01 + 64, kt // 4,
                                   128 * (kt % 4):128 * (kt % 4) + 128],
                                q_sb[64 * h01:64 * h01 + 64, hp, :],
                                start=True, stop=True)
                        p2 = pbuf.tile([128, 2, TOK], bf16, tag="p")
                        nc.scalar.activation(p2[:], s2[:], EXP, scale=0.125)
                        for h01 in range(2):
                            nc.vector.tensor_mul(
                                p2[:, h01, :], p2[:, h01, :],
                                masks_sb[:, kt, :])
                            nc.tensor.matmul(
                                o_pss[h01][:], vv[:, kt, 2 * hp + h01, :],
                                p2[:, h01, :],
                                start=(kt == 0), stop=(kt == NKT - 1))
                    for h01 in range(2):
                        o_ps = o_pss[h01]
                        rrow = rows.tile([1, TOK], f32, tag="rrow", bufs=2)
                        nc.vector.reciprocal(rrow[:], o_ps[HD:HD + 1, :])
                        nb_ps = ps_big.tile([64, TOK], f32, tag="big")
                        nc.tensor.matmul(nb_ps[:], ones_row[:, 0:64], rrow[:],
                                         start=True, stop=True)
                        nb_sb = tmp.tile([64, TOK], f32, tag="nb")
                        nc.vector.tensor_copy(nb_sb[:], nb_ps[:])
                        nc.vector.tensor_mul(
                            o_sb[64 * h01:64 * h01 + 64, hp, :],
                            o_ps[0:HD, :], nb_sb[:])

                # ---- out projection + residual
                for oc in range(DC):
                    woc = wstream.tile([128, DC, 128], bf16, tag="wchunk",
                                       name=f"woc{l}_{oc}")
                    nc.sync.dma_start(
                        woc[:], wout_d[l][:, 128 * oc:128 * (oc + 1)]
                        .rearrange("(c p) n -> p c n", p=128))
                    pp = ps_big.tile([128, TOK], f32, tag="big")
                    for dc in range(DC):
                        nc.tensor.matmul(
                            pp[:], woc[:, dc, :], o_sb[:, dc, :],
                            start=(dc == 0), stop=(dc == DC - 1))
                    nc.vector.tensor_add(x_fm[:, oc, :], pp[:], x_fm[:, oc, :])

                # ---- LN2 + FFN up + silu
                m2b2, rstd_bc2, _r2, _mu2 = ln_stats()
                xb2 = cast_x()
                s_sb = act.tile([128, DFC, TOK], bf16, tag="s_silu")
                for ocp in range(DFC // 2):
                    wc, cs = wcol_chunk(wup_d[l][:, 256 * ocp:256 * (ocp + 1)],
                                        n=256)
                    for k2 in range(2):
                        oc = 2 * ocp + k2
                        sl = slice(128 * k2, 128 * (k2 + 1))
                        pp = ps_big.tile([128, TOK], f32, tag="big")
                        for dc in range(DC):
                            nc.tensor.matmul(pp[:], wc[:, dc, sl],
                                             xb2[:, dc, :],
                                             start=(dc == 0), stop=False)
                        nc.tensor.matmul(pp[:], cs[:, sl], m2b2[:],
                                         start=False, stop=True)
                        ut = tmp.tile([128, TOK], f32, tag="u", bufs=3)
                        nc.vector.tensor_mul(ut[:], pp[:], rstd_bc2[:])
                        nc.scalar.activation(s_sb[:, oc, :], ut[:], SILU)

                # ---- FFN down + residual (single pass, 6 accumulators)
                s2a = ps_big.tile([128, 2, TOK], f32, tag="s2",
                                  name=f"dn_s2a_{l}")
                s2b = ps_big.tile([128, 2, TOK], f32, tag="s2",
                                  name=f"dn_s2b_{l}")
                pb0 = ps_big.tile([128, TOK], f32, tag="big",
                                  name=f"dn_pb0_{l}")
                pb1 = ps_big.tile([128, TOK], f32, tag="big",
                                  name=f"dn_pb1_{l}")
                accs = [s2a[:, 0, :], s2a[:, 1, :], s2b[:, 0, :],
                        s2b[:, 1, :], pb0[:], pb1[:]]
                for dfc in range(DFC):
                    wd_sb = wstream.tile([128, D], bf16, tag="wdn",
                                         name=f"wd{l}_{dfc}")
                    nc.sync.dma_start(wd_sb[:], wdn_d[l, 128 * dfc:
                                                      128 * (dfc + 1), :])
                    for oc in range(DC):
                        nc.tensor.matmul(
                            accs[oc], wd_sb[:, 128 * oc:128 * (oc + 1)],
                            s_sb[:, dfc, :], start=(dfc == 0),
                            stop=(dfc == DFC - 1))
                for oc in range(DC):
                    nc.vector.tensor_add(x_fm[:, oc, :], accs[oc],
                                         x_fm[:, oc, :])

            # ---------------- final LN + AllGather of hidden states ----------
            m2bf, rstd_bcf, rstd_rowf, mu_rowf = ln_stats()
            xh_sb = act.tile([128, DC, TOK], bf16, tag="q")
            for dc in range(DC):
                nc.vector.tensor_mul(xh_sb[:, dc, :], x_fm[:, dc, :],
                                     rstd_bcf[:])
            murs = rows.tile([1, TOK], f32, tag="murs")
            nc.vector.tensor_mul(murs[:], mu_rowf[:], rstd_rowf[:])
            m2p = rows.tile([1, TOK], bf16, tag="m2p")
            nc.vector.tensor_scalar_mul(m2p[:], murs[:], -1.0)
            nc.sync.dma_start(
                xh_in[0:D, :].rearrange("(c p) f -> p c f", p=128), xh_sb[:])
            nc.sync.dma_start(xh_in[D:D + 1, :], m2p[:])
            nc.gpsimd.collective_compute(
                "AllGather", mybir.AluOpType.bypass,
                replica_groups=GROUPS8, ins=[xh_in[:]], outs=[xh_out[:]])

        # ---------------- head phase (separate pool scope) ----------------
        with contextlib.ExitStack() as ctx:
            const2 = ctx.enter_context(tc.tile_pool(name="const2", bufs=1))
            hw = ctx.enter_context(tc.tile_pool(name="hw", bufs=1))
            lg = ctx.enter_context(tc.tile_pool(name="lg", bufs=4))
            rows2 = ctx.enter_context(tc.tile_pool(name="rows2", bufs=2))
            ps_big2 = ctx.enter_context(
                tc.tile_pool(name="ps_big2", bufs=3, space="PSUM"))
            ps_row2 = ctx.enter_context(
                tc.tile_pool(name="ps_row2", bufs=2, space="PSUM"))

            ones_col_b2 = const2.tile([128, 1], bf16, tag="ones_col_b2")
            nc.vector.memset(ones_col_b2[:], 1.0)

            teT_sb = hw.tile([128, DC, VPAD], bf16, tag="teT")
            nc.sync.dma_start(
                teT_sb[:], teT_d[:].rearrange("(c p) n -> p c n", p=128))
            xf_sb = hw.tile([128, DC, NCORES * TOK], bf16, tag="xf")
            m2_sb = rows2.tile([1, NCORES * TOK], bf16, tag="m2")
            for r in range(NCORES):
                base = (D + 1) * r
                for dc in range(DC):
                    nc.sync.dma_start(
                        xf_sb[:, dc, TOK * r:TOK * (r + 1)],
                        xh_out[base + 128 * dc:base + 128 * (dc + 1), :])
                nc.sync.dma_start(m2_sb[:, TOK * r:TOK * (r + 1)],
                                  xh_out[base + D:base + D + 1, :])

            # colsums of teT shard
            chead = rows2.tile([1, VPAD], bf16, tag="chead")
            for vc in range(VPAD // 512):
                cps = ps_row2.tile([1, 512], f32, tag="row2")
                for dc in range(DC):
                    nc.tensor.matmul(cps[:], ones_col_b2[:],
                                     teT_sb[:, dc, 512 * vc:512 * (vc + 1)],
                                     start=(dc == 0), stop=(dc == DC - 1))
                nc.vector.tensor_copy(chead[:, 512 * vc:512 * (vc + 1)],
                                      cps[:])

            for tokc in range(0 if ABLATE == "head" else NCORES * TOK // 128):
                t0 = 128 * tokc
                for vc2 in range(VPAD // 1024):
                    pp = ps_big2.tile([128, 2, 512], f32, tag="big2")
                    for j in range(2):
                        vc = 2 * vc2 + j
                        for dc in range(DC):
                            nc.tensor.matmul(
                                pp[:, j, :], xf_sb[:, dc, t0:t0 + 128],
                                teT_sb[:, dc, 512 * vc:512 * (vc + 1)],
                                start=(dc == 0), stop=False)
                        nc.tensor.matmul(pp[:, j, :], m2_sb[:, t0:t0 + 128],
                                         chead[:, 512 * vc:512 * (vc + 1)],
                                         start=False, stop=True)
                    lg_sb = lg.tile([128, 2, 512], f32, tag="lg")
                    if vc2 % 2 == 0:
                        nc.vector.tensor_copy(lg_sb[:], pp[:])
                    else:
                        nc.scalar.copy(lg_sb[:], pp[:])
                    nc.sync.dma_start(
                        out_d[t0:t0 + 128, 1024 * vc2:1024 * (vc2 + 1)],
                        lg_sb[:].rearrange("p a b -> p (a b)"))

    nc.compile()
    return nc


def _make_runner(nc):
    import jax
    import jax.numpy as jnp
    from jax.sharding import Mesh, PartitionSpec, NamedSharding
    from jax.experimental.shard_map import shard_map
    from concourse import bass2jax, mybir

    bass2jax.install_neuronx_cc_hook()
    partition_name = (nc.partition_id_tensor.name
                      if nc.partition_id_tensor else None)

    SHARED = {"te", "wqkv", "wout", "wup", "wdn"}
    in_names, out_names, out_avals = [], [], []
    for alloc in nc.m.functions[0].allocations:
        if not isinstance(alloc, mybir.MemoryLocationSet):
            continue
        name = alloc.memorylocations[0].name
        if alloc.kind == "ExternalInput":
            if name != partition_name:
                in_names.append(name)
        elif alloc.kind == "ExternalOutput":
            out_names.append(name)
            out_avals.append(jax.core.ShapedArray(
                tuple(alloc.tensor_shape), mybir.dt.np(alloc.dtype)))
    n_params = len(in_names)
    full_names = list(in_names) + list(out_names)
    if partition_name is not None:
        full_names.append(partition_name)

    def _body(*args):
        operands = list(args)
        if partition_name is not None:
            operands.append(bass2jax.partition_id_tensor())
        outs = bass2jax._bass_exec_p.bind(
            *operands,
            out_avals=tuple(out_avals),
            in_names=tuple(full_names),
            out_names=tuple(out_names),
            lowering_input_output_aliases=(),
            sim_require_finite=True,
            sim_require_nnan=True,
            nc=nc,
        )
        return tuple(outs)

    devices = jax.devices()[:NCORES]
    mesh = Mesh(np.asarray(devices), ("core",))
    in_specs = tuple(
        PartitionSpec() if n in SHARED else PartitionSpec("core")
        for n in in_names) + (PartitionSpec("core"),) * len(out_names)
    out_specs = (PartitionSpec("core"),) * len(out_names)
    donate = tuple(range(n_params, n_params + len(out_names)))
    sharded = jax.jit(
        shard_map(_body, mesh=mesh, in_specs=in_specs, out_specs=out_specs,
                  check_rep=False),
        donate_argnums=donate, keep_unused=True)

    sharded_nodonate = jax.jit(
        shard_map(_body, mesh=mesh, in_specs=in_specs, out_specs=out_specs,
                  check_rep=False),
        keep_unused=True)

    shard8 = NamedSharding(mesh, PartitionSpec("core"))
    repl = NamedSharding(mesh, PartitionSpec())

    zfns = [
        jax.jit(
            (lambda av: (lambda: jnp.zeros((NCORES * av.shape[0],)
                                           + av.shape[1:], av.dtype)))(av),
            out_shardings=shard8)
        for av in out_avals
    ]

    def put_inputs(per_core_maps, shared_map):
        dev = []
        for n in in_names:
            if n in SHARED:
                dev.append(jax.device_put(shared_map[n], repl))
            else:
                arr = np.concatenate([m[n] for m in per_core_maps], axis=0)
                dev.append(jax.device_put(arr, shard8))
        return dev

    def run(dev_inputs):
        zeros = [zf() for zf in zfns]
        outs = sharded(*dev_inputs, *zeros)
        jax.block_until_ready(outs)
        return {n: outs[i] for i, n in enumerate(out_names)}

    def run_burst(dev_inputs, n):
        """Enqueue n executions back-to-back (no donation, constant
        buffers), block once.  Wall-time difference between bursts isolates
        per-execution device time from dispatch overhead."""
        zeros = [zf() for zf in zfns]
        jax.block_until_ready(zeros)
        t0 = time.time()
        outs = None
        for _ in range(n):
            outs = sharded_nodonate(*dev_inputs, *zeros)
        jax.block_until_ready(outs)
        return time.time() - t0

    return put_inputs, run, run_burst


def _prepare_inputs(ids, te, pe):
    bf = ml_dtypes.bfloat16
    shared = _STATE["shared"]
    ids = np.asarray(ids)
    per_core = []
    for c in range(NCORES):
        b, cc = c // 4, c % 4
        sl = slice(TOK * cc, TOK * (cc + 1))
        idx = ids[b, sl].astype(np.int32).reshape(TOK, 1)
        pe_s = np.asarray(pe[sl], dtype=np.float32)
        # causal masks: mask[kt][i, j] = 1 if (128*kt + i) <= (512*cc + j)
        ki = (128 * np.arange(NKT)[:, None, None]
              + np.arange(128)[None, :, None])
        qj = TOK * cc + np.arange(TOK)[None, None, :]
        masks = (ki <= qj).astype(bf)
        teT_s = np.zeros((D, VPAD), dtype=bf)
        teT_s[:, :VSH] = te[VSH * c:VSH * (c + 1), :].T.astype(bf)
        per_core.append({"idx": idx, "pe_s": pe_s, "masks": masks,
                         "teT_s": teT_s})
    shared_map = {"te": np.asarray(te, dtype=np.float32), **shared}
    return per_core, shared_map


def kernel(ids, te, pe, ln1_s, ln1_b, qkv_w, qkv_b, out_w, out_b,
           ln2_s, ln2_b, up_w, up_b, dn_w, dn_b, lnf_s, lnf_b):
    bf = ml_dtypes.bfloat16
    # this kernel folds the layernorms into the projections, which relies on
    # identity LN affine params and zero projection biases (true for this
    # model family's init)
    for z in (ln1_b, ln2_b, lnf_b, qkv_b, out_b, up_b, dn_b):
        assert not np.asarray(z).any(), "nonzero bias unsupported"
    for o in (ln1_s, ln2_s, lnf_s):
        assert np.all(np.asarray(o) == 1.0), "non-identity LN scale unsupported"

    if "run" not in _STATE:
        _STATE["shared"] = {
            "wqkv": np.ascontiguousarray(np.asarray(qkv_w)).astype(bf),
            "wout": np.ascontiguousarray(np.asarray(out_w)).astype(bf),
            "wup": np.ascontiguousarray(np.asarray(up_w)).astype(bf),
            "wdn": np.ascontiguousarray(np.asarray(dn_w)).astype(bf),
        }
        nc = _build_program()
        put_inputs, run, run_burst = _make_runner(nc)
        _STATE["put_inputs"] = put_inputs
        _STATE["run"] = run
        _STATE["run_burst"] = run_burst

    per_core, shared_map = _prepare_inputs(ids, te, pe)
    dev_inputs = _STATE["put_inputs"](per_core, shared_map)
    _STATE["dev_inputs"] = dev_inputs
    outs = _STATE["run"](dev_inputs)
    logits = np.asarray(outs["logits"])  # [8*4096, 4096]
    logits = logits.reshape(NCORES, NCORES * TOK, VPAD)[:, :, :VSH]
    # core c rows: [b0 tokens 0..2047, b1 tokens 0..2047]; vocab shard c
    full = np.concatenate([logits[c] for c in range(NCORES)], axis=1)
    return full.reshape(B, T, V).astype(np.float32)



# revision 19
# speedup vs baseline: 1.3494x; 1.3494x over previous
"""Bass/Tile TRN2 kernel for a 4-layer dense transformer (D=768, H=12, DF=3072,
V=32000, B=2, T=2048) sharded across 8 NeuronCores.

Sharding: each core owns 512 tokens (core c -> batch c//4, tokens 512*(c%4)...).
Per layer the LN1-normalized hidden states are AllGathered across the 4-core
batch group (split into two token-half collectives so compute overlaps), and
every core computes K/V for all 2048 context tokens locally -- one small
collective per layer instead of shipping K and V.

Key chunks are processed in a per-core slot order: slots 0-3 are the core's
own (causally diagonal) chunks, computable before any collective; slots 4-15
are peer chunks.  Causal masking uses static relative masks on the diagonal
slots plus a per-core additive bias table on the exp (fully-masked chunks get
-1e4 so exp underflows to zero) -- no elementwise mask is needed off-diagonal.

For the tied LM head the final hidden states are AllGathered across the batch
group in two halves; the vocabulary is sharded V/4=8000 (padded 8192) per
core.  Head output rows are written in slot order (own tokens first, enabling
compute during the gather) and reordered on the host.

Everything numerical is bf16/f32 (fp8 attention was measured to breach the
2e-2 tolerance).  Layernorms are materialized explicitly (normalized copies),
so no projection corrections are needed anywhere.
"""

import os
import sys
import time

for _p in ("/opt/trn_rl_repo", "/root/.axon_site/_ro/trn_rl_repo"):
    if os.path.isdir(_p) and _p not in sys.path:
        sys.path.insert(0, _p)

import numpy as np
import ml_dtypes

D, DF, H, L, V, T_MAX = 768, 3072, 12, 4, 32000, 2048
HD = D // H          # 64
B, T = 2, 2048
NCORES = 8
TOK = 512            # tokens per core
GTOK = 4 * TOK       # tokens per batch group
DC = D // 128        # 6 feature chunks
DFC = DF // 128      # 24
VSH = V // 4         # 8000 vocab per core (sharded within batch group)
VPAD = 8192          # padded vocab shard
NKT = 16             # key chunks of 128 (full 2048 context)
EPS = 1e-5

_STATE = {}
ABLATE = os.environ.get("KERNEL_ABLATE", "")


def _build_program():
    import concourse.bass as bass
    import concourse.tile as tile
    from concourse import bacc, mybir
    from concourse.masks import make_identity

    f32 = mybir.dt.float32
    bf16 = mybir.dt.bfloat16
    i32 = mybir.dt.int32
    EXP = mybir.ActivationFunctionType.Exp
    SILU = mybir.ActivationFunctionType.Silu
    SQRT = mybir.ActivationFunctionType.Sqrt

    nc = bacc.Bacc("TRN2", target_bir_lowering=False, debug=False,
                   num_devices=NCORES)

    # ---------------- DRAM I/O ----------------
    te_d = nc.dram_tensor("te", [V, D], f32, kind="ExternalInput")
    wqkv_d = nc.dram_tensor("wqkv", [L, D, 3 * D], bf16, kind="ExternalInput")
    wout_d = nc.dram_tensor("wout", [L, D, D], bf16, kind="ExternalInput")
    wup_d = nc.dram_tensor("wup", [L, D, DF], bf16, kind="ExternalInput")
    wdn_d = nc.dram_tensor("wdn", [L, DF, D], bf16, kind="ExternalInput")
    # per-core
    idx_d = nc.dram_tensor("idx", [TOK, 1], i32, kind="ExternalInput")
    pe_d = nc.dram_tensor("pe_s", [TOK, D], f32, kind="ExternalInput")
    mrel_d = nc.dram_tensor("mrel", [4, 128, TOK], bf16, kind="ExternalInput")
    btab_d = nc.dram_tensor("btab", [128, NKT], f32, kind="ExternalInput")
    gofs_d = nc.dram_tensor("gofs", [128, DC * 3], i32, kind="ExternalInput")
    teT_d = nc.dram_tensor("teT_s", [D, VPAD], bf16, kind="ExternalInput")
    # output
    out_d = nc.dram_tensor("logits", [GTOK, VPAD], f32, kind="ExternalOutput")

    # internal DRAM for collectives (xn halves per layer, xf halves at end)
    XSPLIT = [(0, 256), (256, 256)]
    xg_in = [nc.dram_tensor(f"xg{i}_in", [D, w], bf16)
             for i, (_, w) in enumerate(XSPLIT)]
    xg_out = [nc.dram_tensor(f"xg{i}_out", [4 * D, w], bf16)
              for i, (_, w) in enumerate(XSPLIT)]
    xh_in = [nc.dram_tensor(f"xh{i}_in", [D, TOK // 2], bf16)
             for i in range(2)]
    xh_out = [nc.dram_tensor(f"xh{i}_out", [4 * D, TOK // 2], bf16)
              for i in range(2)]

    GROUPS4 = [[0, 1, 2, 3], [4, 5, 6, 7]]

    with tile.TileContext(nc) as tc:
        import contextlib
        with tc.tile_pool(name="xfp", bufs=1) as xfp, \
                tc.tile_pool(name="cstp", bufs=1) as cstp:
          with contextlib.ExitStack() as ctx:
            # ---------------- pools ----------------
            const = ctx.enter_context(tc.tile_pool(name="const", bufs=1))
            xp = ctx.enter_context(tc.tile_pool(name="xp", bufs=1))
            act = ctx.enter_context(tc.tile_pool(name="act", bufs=1))
            wstream = ctx.enter_context(tc.tile_pool(name="wstream", bufs=3))
            rows = ctx.enter_context(tc.tile_pool(name="rows", bufs=1))
            tmp = ctx.enter_context(tc.tile_pool(name="tmp", bufs=2))
            pbuf = ctx.enter_context(tc.tile_pool(name="pbuf", bufs=4))
            kkp = ctx.enter_context(tc.tile_pool(name="kkp", bufs=2))
            ps_big = ctx.enter_context(
                tc.tile_pool(name="ps_big", bufs=2, space="PSUM"))
            ps_att = ctx.enter_context(
                tc.tile_pool(name="ps_att", bufs=2, space="PSUM"))
            ps_row = ps_att

            # ---------------- constants ----------------
            ones_col = const.tile([128, 1], f32, tag="ones_col")
            nc.vector.memset(ones_col[:], 1.0)
            ones_row = const.tile([1, 128], f32, tag="ones_row")
            nc.vector.memset(ones_row[:], 1.0)
            eps_t = const.tile([1, 1], f32, tag="eps")
            nc.vector.memset(eps_t[:], EPS)
            ident = const.tile([128, 128], f32, tag="ident")
            make_identity(nc, ident[:])
            mrel_sb = const.tile([128, 4, TOK], bf16, tag="mrel")
            nc.sync.dma_start(mrel_sb[:], mrel_d[:].rearrange("t p f -> p t f"))
            btab_sb = const.tile([128, NKT], f32, tag="btab")
            nc.sync.dma_start(btab_sb[:], btab_d[:])
            gofs_sb = cstp.tile([128, DC * 3], i32, tag="gofs")
            nc.sync.dma_start(gofs_sb[:], gofs_d[:])

            # persistent activations
            x_fm = xp.tile([128, DC, TOK], f32, tag="x_fm")
            xf_sb = xfp.tile([128, DC, TOK], bf16, tag="xf")

            # ---------------- embedding ----------------
            idx_sb = tmp.tile([128, 4, 1], i32, tag="idx")
            nc.sync.dma_start(
                idx_sb[:], idx_d[:].rearrange("(tc p) o -> p tc o", p=128))
            emb_ts = []
            for tc4 in range(4):
                emb_t = tmp.tile([128, D], f32, tag="emb", bufs=3,
                                 name=f"emb{tc4}")
                nc.gpsimd.indirect_dma_start(
                    out=emb_t[:], out_offset=None, in_=te_d[:],
                    in_offset=bass.IndirectOffsetOnAxis(
                        ap=idx_sb[:, tc4, 0:1], axis=0))
                emb_ts.append(emb_t)
            for tc4 in range(4):
                emb_t = emb_ts[tc4]
                pe_t = tmp.tile([128, D], f32, tag="pe", bufs=1)
                nc.sync.dma_start(pe_t[:], pe_d[128 * tc4:128 * (tc4 + 1), :])
                nc.vector.tensor_add(emb_t[:], emb_t[:], pe_t[:])
                for dc in range(DC):
                    tp = ps_att.tile([128, 128], f32, tag="att")
                    nc.tensor.transpose(
                        tp[:], emb_t[:, 128 * dc:128 * (dc + 1)], ident[:])
                    nc.vector.tensor_copy(
                        x_fm[:, dc, 128 * tc4:128 * (tc4 + 1)], tp[:])

            # ---------------- helpers ----------------
            def ln_new():
                sum_ps = ps_row.tile([1, TOK], f32, tag="att")
                sq_ps = ps_row.tile([1, TOK], f32, tag="att")
                return sum_ps, sq_ps

            def ln_feed(st, dc):
                sum_ps, sq_ps = st
                nc.tensor.matmul(sum_ps[:], ones_col[:], x_fm[:, dc, :],
                                 start=(dc == 0), stop=(dc == DC - 1))
                xsq = tmp.tile([128, TOK], f32, tag="xsq", bufs=1)
                nc.vector.tensor_mul(xsq[:], x_fm[:, dc, :], x_fm[:, dc, :])
                nc.tensor.matmul(sq_ps[:], ones_col[:], xsq[:],
                                 start=(dc == 0), stop=(dc == DC - 1))

            def ln_finish(st, dst, ship=None):
                sum_ps, sq_ps = st
                mu_row = rows.tile([1, TOK], f32, tag="mu")
                nc.vector.tensor_scalar_mul(mu_row[:], sum_ps[:], 1.0 / D)
                ex2 = rows.tile([1, TOK], f32, tag="ex2")
                nc.vector.tensor_scalar_mul(ex2[:], sq_ps[:], 1.0 / D)
                var = rows.tile([1, TOK], f32, tag="var")
                nc.vector.tensor_mul(var[:], mu_row[:], mu_row[:])
                nc.vector.tensor_sub(var[:], ex2[:], var[:])
                std = rows.tile([1, TOK], f32, tag="std")
                nc.scalar.activation(std[:], var[:], SQRT, bias=eps_t[:])
                rstd_row = rows.tile([1, TOK], f32, tag="rstd")
                nc.vector.reciprocal(rstd_row[:], std[:])
                bc_ps = ps_big.tile([128, TOK], f32, tag="big")
                nc.tensor.matmul(bc_ps[:], ones_row[:], rstd_row[:],
                                 start=True, stop=True)
                rstd_bc = rows.tile([128, TOK], f32, tag="rstd_bc")
                nc.vector.tensor_copy(rstd_bc[:], bc_ps[:])
                mb_ps = ps_big.tile([128, TOK], f32, tag="big")
                nc.tensor.matmul(mb_ps[:], ones_row[:], mu_row[:],
                                 start=True, stop=True)
                mu_bc = rows.tile([128, TOK], f32, tag="mu_bc")
                nc.vector.tensor_copy(mu_bc[:], mb_ps[:])
                for i in range(2):
                    cs = slice(256 * i, 256 * (i + 1))
                    for dc in range(DC):
                        xc = tmp.tile([128, 256], f32, tag="xsq", bufs=1)
                        nc.vector.tensor_sub(xc[:], x_fm[:, dc, cs],
                                             mu_bc[:, cs])
                        nc.vector.tensor_mul(dst[:, dc, cs], xc[:],
                                             rstd_bc[:, cs])
                    if ship is not None:
                        ship(i)

            _uid = [0]
            _STATS = []

            def wcol_chunk(src_ap, n):
                wc = wstream.tile([128, DC, n], bf16, tag="wchunk",
                                  name=f"wc{_uid[0]}")
                _uid[0] += 1
                nc.sync.dma_start(
                    wc[:], src_ap.rearrange("(c p) n -> p c n", p=128))
                return wc

            xn = act.tile([128, DC, TOK], bf16, tag="xn")
            xg = act.tile([128, DC, 3, TOK], bf16, tag="xg")
            q_sb = act.tile([128, DC, TOK], bf16, tag="q")
            k_own = act.tile([128, DC, TOK], bf16, tag="k_own")
            vv = act.tile([128, NKT, H, HD + 1], bf16, tag="vv")
            o_sb = act.tile([128, DC, TOK], bf16, tag="o")
            o_part = act.tile([HD + 1, DC, 2, TOK], bf16, tag="o_part")
            s_sb = act.tile([128, DFC, TOK], bf16, tag="s_silu")

            # ---------------- layers ----------------
            for l in range(L):
                # ---- LN1 -> xn; ship each half as soon as it is ready
                def _ship_xg(i):
                    o0, w = XSPLIT[i]
                    nc.sync.dma_start(
                        xg_in[i][:].rearrange("(c p) f -> p c f", p=128),
                        xn[:, :, o0:o0 + w])
                    nc.gpsimd.collective_compute(
                        "AllGather", mybir.AluOpType.bypass,
                        replica_groups=GROUPS4, ins=[xg_in[i][:]],
                        outs=[xg_out[i][:]])

                if l == 0:
                    st1 = ln_new()
                    for dc in range(DC):
                        ln_feed(st1, dc)
                else:
                    st1 = _STATS.pop()
                ln_finish(st1, xn, ship=_ship_xg)

                # ---- own projections (overlap the gathers)
                def proj_own(dst, base):
                    for ocp in range(3):
                        col0 = base + 256 * ocp
                        wc = wcol_chunk(wqkv_d[l][:, col0:col0 + 256], n=256)
                        for k2 in range(2):
                            sl = slice(128 * k2, 128 * (k2 + 1))
                            pp = ps_big.tile([128, TOK], f32, tag="big")
                            for dc in range(DC):
                                nc.tensor.matmul(pp[:], wc[:, dc, sl],
                                                 xn[:, dc, :],
                                                 start=(dc == 0),
                                                 stop=(dc == DC - 1))
                            nc.vector.tensor_copy(dst[:, 2 * ocp + k2, :],
                                                  pp[:])

                wk_all = wstream.tile([128, DC, D], bf16, tag="wkall",
                                      name=f"wka{l}", bufs=1)
                nc.sync.dma_start(
                    wk_all[:], wqkv_d[l][:, D:2 * D]
                    .rearrange("(c p) n -> p c n", p=128))
                proj_own(q_sb, 0)
                for oc in range(DC):
                    pp = ps_big.tile([128, TOK], f32, tag="big",
                                     name=f"ko{l}_{oc}")
                    for dc in range(DC):
                        nc.tensor.matmul(
                            pp[:], wk_all[:, dc, 128 * oc:128 * (oc + 1)],
                            xn[:, dc, :], start=(dc == 0),
                            stop=(dc == DC - 1))
                    nc.vector.tensor_copy(k_own[:, oc, :], pp[:])

                nc.vector.memset(vv[:, :, :, HD:HD + 1], 1.0)
                for nv in range(2):
                    col0 = 2 * D + 384 * nv
                    wv = wcol_chunk(wqkv_d[l][:, col0:col0 + 384], n=384)
                    for tc4 in range(4):
                        pp = ps_big.tile([128, 384], f32, tag="big")
                        for dc in range(DC):
                            nc.tensor.matmul(
                                pp[:], xn[:, dc, 128 * tc4:128 * (tc4 + 1)],
                                wv[:, dc, :], start=(dc == 0),
                                stop=(dc == DC - 1))
                        nc.vector.tensor_copy(
                            vv[:, tc4, 6 * nv:6 * (nv + 1), 0:HD],
                            pp[:].rearrange("p (h w) -> p h w", h=6))

                # ---- own-chunk attention (no collective dependency)
                def sc_exp(kk_ap, p2, s, hp, mask_t=None):
                    s2 = ps_big.tile([128, 2, TOK], f32, tag="s2")
                    for h01 in range(2):
                        nc.tensor.matmul(
                            s2[:, h01, :], kk_ap[64 * h01:64 * h01 + 64, :],
                            q_sb[64 * h01:64 * h01 + 64, hp, :],
                            start=True, stop=True)
                    nc.scalar.activation(p2[:], s2[:], EXP, scale=0.125,
                                         bias=btab_sb[:, s:s + 1])
                    if mask_t is not None:
                        for h01 in range(2):
                            nc.vector.tensor_mul(p2[:, h01, :], p2[:, h01, :],
                                                 mrel_sb[:, mask_t, :])

                if ABLATE == "attn":
                    nc.vector.memset(o_sb[:], 0.001)
                    nc.vector.memset(o_part[:], 0.001)
                else:
                    for hp in range(DC):
                        o_psA = ps_att.tile([HD + 1, TOK], f32, tag="att",
                                            name=f"oownA{l}_{hp}")
                        o_psB = ps_att.tile([HD + 1, TOK], f32, tag="att",
                                            name=f"oownB{l}_{hp}")
                        o_pss = (o_psA, o_psB)
                        for t in range(4):
                            p2 = pbuf.tile([128, 2, TOK], bf16, tag="p")
                            sc_exp(k_own[:, hp, 128 * t:128 * (t + 1)],
                                   p2[:], t, hp, mask_t=t)
                            for h01 in range(2):
                                nc.tensor.matmul(
                                    o_pss[h01][:],
                                    vv[:, t, 2 * hp + h01, :], p2[:, h01, :],
                                    start=(t == 0), stop=(t == 3))
                        for h01 in range(2):
                            nc.vector.tensor_copy(o_part[:, hp, h01, :],
                                                  o_pss[h01][:])

                # ---- gather peer xn (after AG), first chunk first
                for i, (o0, w) in enumerate(XSPLIT):
                    for dc in range(DC):
                        for pj in range(3):
                            nc.gpsimd.indirect_dma_start(
                                out=xg[:, dc, pj, o0:o0 + w],
                                out_offset=None, in_=xg_out[i][:],
                                in_offset=bass.IndirectOffsetOnAxis(
                                    ap=gofs_sb[:, 3 * dc + pj:
                                               3 * dc + pj + 1], axis=0))

                PH_TCJ = [[0, 1], [2, 3]]
                for half in range(2):
                    # peer V for this phase's token chunks
                    for nv in range(2):
                        col0 = 2 * D + 384 * nv
                        wv = wcol_chunk(wqkv_d[l][:, col0:col0 + 384], n=384)
                        for pj in range(3):
                            for tcj in PH_TCJ[half]:
                                sv = 4 * pj + tcj
                                pp = ps_big.tile([128, 384], f32, tag="big")
                                for dc in range(DC):
                                    nc.tensor.matmul(
                                        pp[:],
                                        xg[:, dc, pj,
                                           128 * tcj:128 * (tcj + 1)],
                                        wv[:, dc, :], start=(dc == 0),
                                        stop=(dc == DC - 1))
                                nc.vector.tensor_copy(
                                    vv[:, 4 + sv, 6 * nv:6 * (nv + 1), 0:HD],
                                    pp[:].rearrange("p (h w) -> p h w", h=6))

                    if ABLATE == "attn":
                        continue
                    o0, w = XSPLIT[half]
                    for hp in range(DC):
                        # K for this phase's peer tokens
                        kk = kkp.tile([128, 3, 256], bf16, tag="kk")
                        for pj in range(3):
                            pp = ps_big.tile([128, 256], f32, tag="big",
                                             name=f"kp{l}_{half}_{hp}_{pj}")
                            for dc in range(DC):
                                nc.tensor.matmul(
                                    pp[:, 0:w],
                                    wk_all[:, dc,
                                           128 * hp:128 * (hp + 1)],
                                    xg[:, dc, pj, o0:o0 + w],
                                    start=(dc == 0), stop=(dc == DC - 1))
                            nc.vector.tensor_copy(kk[:, pj, 0:w],
                                                  pp[:, 0:w])
                        o_psA = ps_att.tile([HD + 1, TOK], f32, tag="att",
                                            name=f"opA{l}_{half}_{hp}")
                        o_psB = ps_att.tile([HD + 1, TOK], f32, tag="att",
                                            name=f"opB{l}_{half}_{hp}")
                        o_pss = (o_psA, o_psB)
                        ntc = len(PH_TCJ[half])
                        for si in range(3 * ntc):
                            pj, t2 = si // ntc, si % ntc
                            tcj = PH_TCJ[half][t2]
                            sv = 4 * pj + tcj
                            p2 = pbuf.tile([128, 2, TOK], bf16, tag="p")
                            sc_exp(kk[:, pj, 128 * t2:128 * (t2 + 1)],
                                   p2[:], 4 + sv, hp)
                            for h01 in range(2):
                                nc.tensor.matmul(
                                    o_pss[h01][:],
                                    vv[:, 4 + sv, 2 * hp + h01, :],
                                    p2[:, h01, :],
                                    start=(si == 0),
                                    stop=(si == 3 * ntc - 1))
                        if half == 0:
                            for h01 in range(2):
                                nc.vector.tensor_add(
                                    o_part[:, hp, h01, :],
                                    o_pss[h01][:], o_part[:, hp, h01, :])
                        else:
                            for h01 in range(2):
                                osum = tmp.tile([HD + 1, TOK], f32,
                                                tag="osum")
                                nc.vector.tensor_add(
                                    osum[:], o_pss[h01][:],
                                    o_part[:, hp, h01, :])
                                rrow = rows.tile([1, TOK], f32, tag="rrow",
                                                 bufs=2)
                                nc.vector.reciprocal(rrow[:],
                                                     osum[HD:HD + 1, :])
                                nb_ps = ps_big.tile([64, TOK], f32,
                                                    tag="big")
                                nc.tensor.matmul(nb_ps[:],
                                                 ones_row[:, 0:64],
                                                 rrow[:], start=True,
                                                 stop=True)
                                nb_sb = tmp.tile([64, TOK], bf16, tag="nb")
                                nc.vector.tensor_copy(nb_sb[:], nb_ps[:])
                                nc.vector.tensor_mul(
                                    o_sb[64 * h01:64 * h01 + 64, hp, :],
                                    osum[0:HD, :], nb_sb[:])

                # ---- out projection + residual (LN2 stats interleaved)
                st2 = ln_new()
                for oc in range(DC):
                    woc = wstream.tile([128, DC, 128], bf16, tag="wk",
                                       name=f"woc{l}_{oc}", bufs=2)
                    nc.sync.dma_start(
                        woc[:], wout_d[l][:, 128 * oc:128 * (oc + 1)]
                        .rearrange("(c p) n -> p c n", p=128))
                    pp = ps_big.tile([128, TOK], f32, tag="big")
                    for dc in range(DC):
                        nc.tensor.matmul(
                            pp[:], woc[:, dc, :], o_sb[:, dc, :],
                            start=(dc == 0), stop=(dc == DC - 1))
                    nc.vector.tensor_add(x_fm[:, oc, :], pp[:], x_fm[:, oc, :])
                    ln_feed(st2, oc)

                # ---- LN2 + FFN up + silu (silu straight from PSUM)
                ln_finish(st2, xn)
                for ocp in range(DFC // 2):
                    wc = wcol_chunk(wup_d[l][:, 256 * ocp:256 * (ocp + 1)],
                                    n=256)
                    for k2 in range(2):
                        oc = 2 * ocp + k2
                        sl = slice(128 * k2, 128 * (k2 + 1))
                        pp = ps_big.tile([128, TOK], f32, tag="big")
                        for dc in range(DC):
                            nc.tensor.matmul(pp[:], wc[:, dc, sl],
                                             xn[:, dc, :],
                                             start=(dc == 0),
                                             stop=(dc == DC - 1))
                        nc.scalar.activation(s_sb[:, oc, :], pp[:], SILU)

                # ---- FFN down + residual (single pass, 6 accumulators)
                s2a = ps_big.tile([128, 2, TOK], f32, tag="s2",
                                  name=f"dn_s2a_{l}")
                s2b = ps_big.tile([128, 2, TOK], f32, tag="s2",
                                  name=f"dn_s2b_{l}")
                pb0 = ps_big.tile([128, TOK], f32, tag="big",
                                  name=f"dn_pb0_{l}")
                pb1 = ps_big.tile([128, TOK], f32, tag="big",
                                  name=f"dn_pb1_{l}")
                accs = [s2a[:, 0, :], s2a[:, 1, :], s2b[:, 0, :],
                        s2b[:, 1, :], pb0[:], pb1[:]]
                for dfc in range(DFC):
                    wd_sb = wstream.tile([128, D], bf16, tag="wdn",
                                         name=f"wd{l}_{dfc}")
                    nc.sync.dma_start(wd_sb[:], wdn_d[l, 128 * dfc:
                                                      128 * (dfc + 1), :])
                    for oc in range(DC):
                        nc.tensor.matmul(
                            accs[oc], wd_sb[:, 128 * oc:128 * (oc + 1)],
                            s_sb[:, dfc, :], start=(dfc == 0),
                            stop=(dfc == DFC - 1))
                stn = ln_new()
                for oc in range(DC):
                    nc.vector.tensor_add(x_fm[:, oc, :], accs[oc],
                                         x_fm[:, oc, :])
                    ln_feed(stn, oc)
                _STATS.append(stn)

            # ---------------- final LN -> xf; 2-half AllGather ----
            def _ship_xh(i):
                nc.sync.dma_start(
                    xh_in[i][:].rearrange("(c p) f -> p c f", p=128),
                    xf_sb[:, :, 256 * i:256 * (i + 1)])
                nc.gpsimd.collective_compute(
                    "AllGather", mybir.AluOpType.bypass,
                    replica_groups=GROUPS4, ins=[xh_in[i][:]],
                    outs=[xh_out[i][:]])

            ln_finish(_STATS.pop(), xf_sb, ship=_ship_xh)

          # ---------------- head phase (separate pool scope) --------------
          with contextlib.ExitStack() as ctx:
            hw = ctx.enter_context(tc.tile_pool(name="hw", bufs=1))
            lg = ctx.enter_context(tc.tile_pool(name="lg", bufs=4))
            ps_big2 = ctx.enter_context(
                tc.tile_pool(name="ps_big2", bufs=3, space="PSUM"))

            # resident vocab-shard embedding (transposed), 16 chunk loads
            teT_sb = hw.tile([128, DC, VPAD], bf16, tag="teT")
            for vc in range(VPAD // 512):
                nc.sync.dma_start(
                    teT_sb[:, :, 512 * vc:512 * (vc + 1)],
                    teT_d[:, 512 * vc:512 * (vc + 1)]
                    .rearrange("(c p) n -> p c n", p=128))

            # peer hidden states, gathered per (half, dc)
            xa = hw.tile([128, DC, 2, 3, 256], bf16, tag="xa")
            for half in range(2):
                for dc in range(DC):
                    for pj in range(3):
                        nc.gpsimd.indirect_dma_start(
                            out=xa[:, dc, half, pj, :],
                            out_offset=None, in_=xh_out[half][:],
                            in_offset=bass.IndirectOffsetOnAxis(
                                ap=gofs_sb[:, 3 * dc + pj:3 * dc + pj + 1],
                                axis=0))

            def head_block(sl, lhsT_fn, ti):
                for vc in range(VPAD // 512):
                    pp = ps_big2.tile([128, 512], f32, tag="big2")
                    for dc in range(DC):
                        nc.tensor.matmul(
                            pp[:], lhsT_fn(dc),
                            teT_sb[:, dc, 512 * vc:512 * (vc + 1)],
                            start=(dc == 0), stop=(dc == DC - 1))
                    lg_sb = lg.tile([128, 512], f32, tag="lg")
                    if (ti + vc) % 2 == 0:
                        nc.vector.tensor_copy(lg_sb[:], pp[:])
                    else:
                        nc.scalar.copy(lg_sb[:], pp[:])
                    nc.sync.dma_start(
                        out_d[128 * sl:128 * (sl + 1),
                              512 * vc:512 * (vc + 1)],
                        lg_sb[:])

            if ABLATE != "head":
                for t in range(4):  # own tokens first
                    head_block(
                        t,
                        (lambda tt: (lambda dc:
                                     xf_sb[:, dc, 128 * tt:128 * (tt + 1)]))(t),
                        t)
                for half in range(2):
                    for s in range(6):
                        pj, tj = s // 2, s % 2
                        head_block(
                            4 + 6 * half + s,
                            (lambda hh, pp_, tt: (lambda dc:
                             xa[:, dc, hh, pp_,
                                128 * tt:128 * (tt + 1)]))(half, pj, tj),
                            s)

    nc.compile()
    return nc


def _make_runner(nc):
    import jax
    import jax.numpy as jnp
    from jax.sharding import Mesh, PartitionSpec, NamedSharding
    from jax.experimental.shard_map import shard_map
    from concourse import bass2jax, mybir

    bass2jax.install_neuronx_cc_hook()
    partition_name = (nc.partition_id_tensor.name
                      if nc.partition_id_tensor else None)

    SHARED = {"te", "wqkv", "wout", "wup", "wdn"}
    in_names, out_names, out_avals = [], [], []
    for alloc in nc.m.functions[0].allocations:
        if not isinstance(alloc, mybir.MemoryLocationSet):
            continue
        name = alloc.memorylocations[0].name
        if alloc.kind == "ExternalInput":
            if name != partition_name:
                in_names.append(name)
        elif alloc.kind == "ExternalOutput":
            out_names.append(name)
            out_avals.append(jax.core.ShapedArray(
                tuple(alloc.tensor_shape), mybir.dt.np(alloc.dtype)))
    n_params = len(in_names)
    full_names = list(in_names) + list(out_names)
    if partition_name is not None:
        full_names.append(partition_name)

    def _body(*args):
        operands = list(args)
        if partition_name is not None:
            operands.append(bass2jax.partition_id_tensor())
        outs = bass2jax._bass_exec_p.bind(
            *operands,
            out_avals=tuple(out_avals),
            in_names=tuple(full_names),
            out_names=tuple(out_names),
            lowering_input_output_aliases=(),
            sim_require_finite=True,
            sim_require_nnan=True,
            nc=nc,
        )
        return tuple(outs)

    devices = jax.devices()[:NCORES]
    mesh = Mesh(np.asarray(devices), ("core",))
    in_specs = tuple(
        PartitionSpec() if n in SHARED else PartitionSpec("core")
        for n in in_names) + (PartitionSpec("core"),) * len(out_names)
    out_specs = (PartitionSpec("core"),) * len(out_names)
    donate = tuple(range(n_params, n_params + len(out_names)))
    sharded = jax.jit(
        shard_map(_body, mesh=mesh, in_specs=in_specs, out_specs=out_specs,
                  check_rep=False),
        donate_argnums=donate, keep_unused=True)

    sharded_nodonate = jax.jit(
        shard_map(_body, mesh=mesh, in_specs=in_specs, out_specs=out_specs,
                  check_rep=False),
        keep_unused=True)

    shard8 = NamedSharding(mesh, PartitionSpec("core"))
    repl = NamedSharding(mesh, PartitionSpec())

    zfns = [
        jax.jit(
            (lambda av: (lambda: jnp.zeros((NCORES * av.shape[0],)
                                           + av.shape[1:], av.dtype)))(av),
            out_shardings=shard8)
        for av in out_avals
    ]

    def put_inputs(per_core_maps, shared_map):
        dev = []
        for n in in_names:
            if n in SHARED:
                dev.append(jax.device_put(shared_map[n], repl))
            else:
                arr = np.concatenate([m[n] for m in per_core_maps], axis=0)
                dev.append(jax.device_put(arr, shard8))
        return dev

    def run(dev_inputs):
        zeros = [zf() for zf in zfns]
        outs = sharded(*dev_inputs, *zeros)
        jax.block_until_ready(outs)
        return {n: outs[i] for i, n in enumerate(out_names)}

    def run_burst(dev_inputs, n):
        zeros = [zf() for zf in zfns]
        jax.block_until_ready(zeros)
        t0 = time.time()
        outs = None
        for _ in range(n):
            outs = sharded_nodonate(*dev_inputs, *zeros)
        jax.block_until_ready(outs)
        return time.time() - t0

    return put_inputs, run, run_burst


def _prepare_inputs(ids, te, pe):
    bf = ml_dtypes.bfloat16
    ids = np.asarray(ids)
    te_f = np.asarray(te, dtype=np.float32)
    per_core = []
    for c in range(NCORES):
        b, cc = c // 4, c % 4
        peers = [r for r in range(4) if r != cc]
        sl = slice(TOK * cc, TOK * (cc + 1))
        idx = ids[b, sl].astype(np.int32).reshape(TOK, 1)
        pe_s = np.asarray(pe[sl], dtype=np.float32)
        # relative diagonal masks: mrel[t][i, j] = 1 if 128*t + i <= j
        ki = (128 * np.arange(4)[:, None, None]
              + np.arange(128)[None, :, None])
        qj = np.arange(TOK)[None, None, :]
        mrel = (ki <= qj).astype(bf)
        # exp bias: own slots 0, peer slot visible iff peer rank < cc
        btab = np.zeros((128, NKT), dtype=np.float32)
        for s in range(12):
            if peers[s // 4] >= cc:
                btab[:, 4 + s] = -30.0
        # gather offsets: row = 768*peer + 128*dc + p
        gofs = np.zeros((128, DC * 3), dtype=np.int32)
        for dc in range(DC):
            for j in range(3):
                gofs[:, 3 * dc + j] = (768 * peers[j] + 128 * dc
                                       + np.arange(128))
        teT_s = np.zeros((D, VPAD), dtype=bf)
        teT_s[:, :VSH] = te_f[VSH * cc:VSH * (cc + 1), :].T.astype(bf)
        if os.environ.get("KERNEL_GOFS0"):
            gofs[:] = 0
        if os.environ.get("KERNEL_BTAB0"):
            btab[:] = 0.0
        if os.environ.get("KERNEL_MREL1"):
            mrel[:] = 1.0
        per_core.append({"idx": idx, "pe_s": pe_s, "mrel": mrel,
                         "btab": btab, "gofs": gofs, "teT_s": teT_s})
    shared_map = {
        "te": te_f,
        "wqkv": _STATE["shared"]["wqkv"],
        "wout": _STATE["shared"]["wout"],
        "wup": _STATE["shared"]["wup"],
        "wdn": _STATE["shared"]["wdn"],
    }
    return per_core, shared_map


def kernel(ids, te, pe, ln1_s, ln1_b, qkv_w, qkv_b, out_w, out_b,
           ln2_s, ln2_b, up_w, up_b, dn_w, dn_b, lnf_s, lnf_b):
    bf = ml_dtypes.bfloat16
    # identity LN affine params and zero biases (true for this model family)
    for z in (ln1_b, ln2_b, lnf_b, qkv_b, out_b, up_b, dn_b):
        assert not np.asarray(z).any(), "nonzero bias unsupported"
    for o in (ln1_s, ln2_s, lnf_s):
        assert np.all(np.asarray(o) == 1.0), "non-identity LN scale unsupported"

    if "run" not in _STATE:
        _STATE["shared"] = {
            "wqkv": np.ascontiguousarray(np.asarray(qkv_w)).astype(bf),
            "wout": np.ascontiguousarray(np.asarray(out_w)).astype(bf),
            "wup": np.ascontiguousarray(np.asarray(up_w)).astype(bf),
            "wdn": np.ascontiguousarray(np.asarray(dn_w)).astype(bf),
        }
        nc = _build_program()
        put_inputs, run, run_burst = _make_runner(nc)
        _STATE["put_inputs"] = put_inputs
        _STATE["run"] = run
        _STATE["run_burst"] = run_burst

    per_core, shared_map = _prepare_inputs(ids, te, pe)
    dev_inputs = _STATE["put_inputs"](per_core, shared_map)
    _STATE["dev_inputs"] = dev_inputs
    outs = _STATE["run"](dev_inputs)
    logits = np.asarray(outs["logits"])  # [8*2048, 8192]
    logits = logits.reshape(NCORES, GTOK, VPAD)[:, :, :VSH]
    # core c = 4b + cc: rows are slot-ordered; map slots back to tokens
    full = np.empty((B, T, V), dtype=np.float32)
    for c in range(NCORES):
        b, cc = c // 4, c % 4
        peers = [r for r in range(4) if r != cc]
        tokmap = np.empty(GTOK, dtype=np.int64)
        for t in range(4):  # own slots
            tokmap[128 * t:128 * (t + 1)] = (TOK * cc + 128 * t
                                             + np.arange(128))
        for half in range(2):
            for s in range(6):
                pj, tj = s // 2, s % 2
                sl_ = 4 + 6 * half + s
                base = TOK * peers[pj] + 256 * half + 128 * tj
                tokmap[128 * sl_:128 * (sl_ + 1)] = base + np.arange(128)
        full[b, tokmap, VSH * cc:VSH * (cc + 1)] = logits[c]
    return full


# revision 24
# speedup vs baseline: 1.4669x; 1.0871x over previous
"""Bass/Tile TRN2 kernel for a 4-layer dense transformer (D=768, H=12, DF=3072,
V=32000, B=2, T=2048) sharded across 8 NeuronCores.

Sharding: each core owns 512 tokens (core c -> batch c//4, tokens 512*(c%4)...).
Per layer the LN1-normalized hidden states are AllGathered across the 4-core
batch group (split into two token-half collectives so compute overlaps), and
every core computes K/V for all 2048 context tokens locally -- one small
collective per layer instead of shipping K and V.

Key chunks are processed in a per-core slot order: slots 0-3 are the core's
own (causally diagonal) chunks, computable before any collective; slots 4-15
are peer chunks.  Causal masking uses static relative masks on the diagonal
slots plus a per-core additive bias table on the exp (fully-masked chunks get
-1e4 so exp underflows to zero) -- no elementwise mask is needed off-diagonal.

For the tied LM head the final hidden states are AllGathered across the batch
group in two halves; the vocabulary is sharded V/4=8000 (padded 8192) per
core.  Head output rows are written in slot order (own tokens first, enabling
compute during the gather) and reordered on the host.

Everything numerical is bf16/f32 (fp8 attention was measured to breach the
2e-2 tolerance).  Layernorms are materialized explicitly (normalized copies),
so no projection corrections are needed anywhere.
"""

import os
import sys
import time

for _p in ("/opt/trn_rl_repo", "/root/.axon_site/_ro/trn_rl_repo"):
    if os.path.isdir(_p) and _p not in sys.path:
        sys.path.insert(0, _p)

import numpy as np
import ml_dtypes

D, DF, H, L, V, T_MAX = 768, 3072, 12, 4, 32000, 2048
HD = D // H          # 64
B, T = 2, 2048
NCORES = 8
TOK = 512            # tokens per core
GTOK = 4 * TOK       # tokens per batch group
DC = D // 128        # 6 feature chunks
DFC = DF // 128      # 24
VSH = V // 4         # 8000 vocab per core (sharded within batch group)
VPAD = 8192          # padded vocab shard
NKT = 16             # key chunks of 128 (full 2048 context)
EPS = 1e-5

_STATE = {}
ABLATE = os.environ.get("KERNEL_ABLATE", "")


def _build_program():
    import concourse.bass as bass
    import concourse.tile as tile
    from concourse import bacc, mybir
    from concourse.masks import make_identity

    f32 = mybir.dt.float32
    bf16 = mybir.dt.bfloat16
    i32 = mybir.dt.int32
    EXP = mybir.ActivationFunctionType.Exp
    SILU = mybir.ActivationFunctionType.Silu
    SQRT = mybir.ActivationFunctionType.Sqrt

    nc = bacc.Bacc("TRN2", target_bir_lowering=False, debug=False,
                   num_devices=NCORES)

    # ---------------- DRAM I/O ----------------
    te_d = nc.dram_tensor("te", [V, D], f32, kind="ExternalInput")
    wqkv_d = nc.dram_tensor("wqkv", [L, D, 3 * D], bf16, kind="ExternalInput")
    wout_d = nc.dram_tensor("wout", [L, D, D], bf16, kind="ExternalInput")
    wup_d = nc.dram_tensor("wup", [L, D, DF], bf16, kind="ExternalInput")
    wdn_d = nc.dram_tensor("wdn", [L, DF, D], bf16, kind="ExternalInput")
    # per-core
    idx_d = nc.dram_tensor("idx", [TOK, 1], i32, kind="ExternalInput")
    pe_d = nc.dram_tensor("pe_s", [TOK, D], f32, kind="ExternalInput")
    mrel_d = nc.dram_tensor("mrel", [4, 128, TOK], bf16, kind="ExternalInput")
    btab_d = nc.dram_tensor("btab", [128, NKT], f32, kind="ExternalInput")
    gofs_d = nc.dram_tensor("gofs", [128, DC * 3], i32, kind="ExternalInput")
    teT_d = nc.dram_tensor("teT_s", [D, VPAD], bf16, kind="ExternalInput")
    # output
    out_d = nc.dram_tensor("logits", [GTOK, VPAD], f32, kind="ExternalOutput")

    # internal DRAM for collectives (xn halves per layer, xf halves at end)
    XSPLIT = [(0, 256), (256, 256)]
    xg_in = [nc.dram_tensor(f"xg{i}_in", [D, w], bf16)
             for i, (_, w) in enumerate(XSPLIT)]
    xg_out = [nc.dram_tensor(f"xg{i}_out", [4 * D, w], bf16)
              for i, (_, w) in enumerate(XSPLIT)]
    xh_in = [nc.dram_tensor(f"xh{i}_in", [D, TOK // 2], bf16)
             for i in range(2)]
    xh_out = [nc.dram_tensor(f"xh{i}_out", [4 * D, TOK // 2], bf16)
              for i in range(2)]

    GROUPS4 = [[0, 1, 2, 3], [4, 5, 6, 7]]

    with tile.TileContext(nc) as tc:
        import contextlib
        with tc.tile_pool(name="xfp", bufs=1) as xfp, \
                tc.tile_pool(name="cstp", bufs=1) as cstp:
          with contextlib.ExitStack() as ctx:
            # ---------------- pools ----------------
            const = ctx.enter_context(tc.tile_pool(name="const", bufs=1))
            xp = ctx.enter_context(tc.tile_pool(name="xp", bufs=1))
            act = ctx.enter_context(tc.tile_pool(name="act", bufs=1))
            wstream = ctx.enter_context(tc.tile_pool(name="wstream", bufs=3))
            rows = ctx.enter_context(tc.tile_pool(name="rows", bufs=1))
            tmp = ctx.enter_context(tc.tile_pool(name="tmp", bufs=2))
            pbuf = ctx.enter_context(tc.tile_pool(name="pbuf", bufs=4))
            kkp = ctx.enter_context(tc.tile_pool(name="kkp", bufs=2))
            ps_big = ctx.enter_context(
                tc.tile_pool(name="ps_big", bufs=2, space="PSUM"))
            ps_att = ctx.enter_context(
                tc.tile_pool(name="ps_att", bufs=2, space="PSUM"))
            ps_row = ps_att

            # ---------------- constants ----------------
            ones_col = const.tile([128, 1], f32, tag="ones_col")
            nc.vector.memset(ones_col[:], 1.0)
            ones_row = const.tile([1, 128], f32, tag="ones_row")
            nc.vector.memset(ones_row[:], 1.0)
            eps_t = const.tile([1, 1], f32, tag="eps")
            nc.vector.memset(eps_t[:], EPS)
            ident = const.tile([128, 128], f32, tag="ident")
            make_identity(nc, ident[:])
            mrel_sb = const.tile([128, 4, TOK], bf16, tag="mrel")
            nc.sync.dma_start(mrel_sb[:], mrel_d[:].rearrange("t p f -> p t f"))
            btab_sb = const.tile([128, NKT], f32, tag="btab")
            nc.sync.dma_start(btab_sb[:], btab_d[:])
            gofs_sb = cstp.tile([128, DC * 3], i32, tag="gofs")
            nc.sync.dma_start(gofs_sb[:], gofs_d[:])

            # persistent activations
            x_fm = xp.tile([128, DC, TOK], f32, tag="x_fm")
            xf_sb = xfp.tile([128, DC, TOK], bf16, tag="xf")

            # ---------------- embedding ----------------
            idx_sb = tmp.tile([128, 4, 1], i32, tag="idx")
            nc.sync.dma_start(
                idx_sb[:], idx_d[:].rearrange("(tc p) o -> p tc o", p=128))
            emb_ts = []
            for tc4 in range(4):
                emb_t = tmp.tile([128, D], f32, tag="emb", bufs=3,
                                 name=f"emb{tc4}")
                nc.gpsimd.indirect_dma_start(
                    out=emb_t[:], out_offset=None, in_=te_d[:],
                    in_offset=bass.IndirectOffsetOnAxis(
                        ap=idx_sb[:, tc4, 0:1], axis=0))
                emb_ts.append(emb_t)
            for tc4 in range(4):
                emb_t = emb_ts[tc4]
                pe_t = tmp.tile([128, D], f32, tag="pe", bufs=1)
                nc.sync.dma_start(pe_t[:], pe_d[128 * tc4:128 * (tc4 + 1), :])
                nc.vector.tensor_add(emb_t[:], emb_t[:], pe_t[:])
                for dc in range(DC):
                    tp = ps_att.tile([128, 128], f32, tag="att")
                    nc.tensor.transpose(
                        tp[:], emb_t[:, 128 * dc:128 * (dc + 1)], ident[:])
                    nc.vector.tensor_copy(
                        x_fm[:, dc, 128 * tc4:128 * (tc4 + 1)], tp[:])

            # ---------------- helpers ----------------
            def ln_new():
                sum_ps = ps_row.tile([1, TOK], f32, tag="att")
                sq_ps = ps_row.tile([1, TOK], f32, tag="att")
                return sum_ps, sq_ps

            def ln_feed(st, dc):
                sum_ps, sq_ps = st
                nc.tensor.matmul(sum_ps[:], ones_col[:], x_fm[:, dc, :],
                                 start=(dc == 0), stop=(dc == DC - 1))
                xsq = tmp.tile([128, TOK], f32, tag="xsq", bufs=1)
                nc.vector.tensor_mul(xsq[:], x_fm[:, dc, :], x_fm[:, dc, :])
                nc.tensor.matmul(sq_ps[:], ones_col[:], xsq[:],
                                 start=(dc == 0), stop=(dc == DC - 1))

            def ln_finish(st, dst, ship=None):
                sum_ps, sq_ps = st
                mu_row = rows.tile([1, TOK], f32, tag="mu")
                nc.vector.tensor_scalar_mul(mu_row[:], sum_ps[:], 1.0 / D)
                ex2 = rows.tile([1, TOK], f32, tag="ex2")
                nc.vector.tensor_scalar_mul(ex2[:], sq_ps[:], 1.0 / D)
                var = rows.tile([1, TOK], f32, tag="var")
                nc.vector.tensor_mul(var[:], mu_row[:], mu_row[:])
                nc.vector.tensor_sub(var[:], ex2[:], var[:])
                std = rows.tile([1, TOK], f32, tag="std")
                nc.scalar.activation(std[:], var[:], SQRT, bias=eps_t[:])
                rstd_row = rows.tile([1, TOK], f32, tag="rstd")
                nc.vector.reciprocal(rstd_row[:], std[:])
                bc_ps = ps_big.tile([128, TOK], f32, tag="big")
                nc.tensor.matmul(bc_ps[:], ones_row[:], rstd_row[:],
                                 start=True, stop=True)
                rstd_bc = rows.tile([128, TOK], f32, tag="rstd_bc")
                nc.vector.tensor_copy(rstd_bc[:], bc_ps[:])
                mb_ps = ps_big.tile([128, TOK], f32, tag="big")
                nc.tensor.matmul(mb_ps[:], ones_row[:], mu_row[:],
                                 start=True, stop=True)
                mu_bc = rows.tile([128, TOK], f32, tag="mu_bc")
                nc.vector.tensor_copy(mu_bc[:], mb_ps[:])
                for i in range(2):
                    cs = slice(256 * i, 256 * (i + 1))
                    for dc in range(DC):
                        xc = tmp.tile([128, 256], f32, tag="xsq", bufs=1)
                        nc.vector.tensor_sub(xc[:], x_fm[:, dc, cs],
                                             mu_bc[:, cs])
                        nc.vector.tensor_mul(dst[:, dc, cs], xc[:],
                                             rstd_bc[:, cs])
                    if ship is not None:
                        ship(i)

            _uid = [0]
            _STATS = []

            def wcol_chunk(src_ap, n):
                wc = wstream.tile([128, DC, n], bf16, tag="wchunk",
                                  name=f"wc{_uid[0]}")
                _uid[0] += 1
                nc.sync.dma_start(
                    wc[:], src_ap.rearrange("(c p) n -> p c n", p=128))
                return wc

            xn = act.tile([128, DC, TOK], bf16, tag="xn")
            xg = act.tile([128, DC, 3, TOK], bf16, tag="xg")
            q_sb = act.tile([128, DC, TOK], bf16, tag="q")
            k_own = act.tile([128, DC, TOK], bf16, tag="k_own")
            vv = act.tile([128, NKT, H, HD + 1], bf16, tag="vv")
            o_sb = act.tile([128, DC, TOK], bf16, tag="o")
            o_part = act.tile([HD + 1, DC, 2, TOK], bf16, tag="o_part")
            s_sb = act.tile([128, DFC, TOK], bf16, tag="s_silu")

            # ---------------- layers ----------------
            for l in range(L):
                # ---- LN1 -> xn; ship each half as soon as it is ready
                def _ship_xg(i):
                    o0, w = XSPLIT[i]
                    nc.sync.dma_start(
                        xg_in[i][:].rearrange("(c p) f -> p c f", p=128),
                        xn[:, :, o0:o0 + w])
                    nc.gpsimd.collective_compute(
                        "AllGather", mybir.AluOpType.bypass,
                        replica_groups=GROUPS4, ins=[xg_in[i][:]],
                        outs=[xg_out[i][:]])

                if l == 0:
                    st1 = ln_new()
                    for dc in range(DC):
                        ln_feed(st1, dc)
                else:
                    st1 = _STATS.pop()
                ln_finish(st1, xn, ship=_ship_xg)

                # ---- own projections (overlap the gathers)
                def proj_own(dst, base):
                    for ocp in range(3):
                        col0 = base + 256 * ocp
                        wc = wcol_chunk(wqkv_d[l][:, col0:col0 + 256], n=256)
                        for k2 in range(2):
                            sl = slice(128 * k2, 128 * (k2 + 1))
                            pp = ps_big.tile([128, TOK], f32, tag="big")
                            for dc in range(DC):
                                nc.tensor.matmul(pp[:], wc[:, dc, sl],
                                                 xn[:, dc, :],
                                                 start=(dc == 0),
                                                 stop=(dc == DC - 1))
                            nc.vector.tensor_copy(dst[:, 2 * ocp + k2, :],
                                                  pp[:])

                wk_all = wstream.tile([128, DC, D], bf16, tag="wkall",
                                      name=f"wka{l}", bufs=1)
                nc.sync.dma_start(
                    wk_all[:], wqkv_d[l][:, D:2 * D]
                    .rearrange("(c p) n -> p c n", p=128))
                proj_own(q_sb, 0)
                for oc in range(DC):
                    pp = ps_big.tile([128, TOK], f32, tag="big",
                                     name=f"ko{l}_{oc}")
                    for dc in range(DC):
                        nc.tensor.matmul(
                            pp[:], wk_all[:, dc, 128 * oc:128 * (oc + 1)],
                            xn[:, dc, :], start=(dc == 0),
                            stop=(dc == DC - 1))
                    nc.vector.tensor_copy(k_own[:, oc, :], pp[:])

                nc.vector.memset(vv[:, :, :, HD:HD + 1], 1.0)
                for nv in range(2):
                    col0 = 2 * D + 384 * nv
                    wv = wcol_chunk(wqkv_d[l][:, col0:col0 + 384], n=384)
                    for tc4 in range(4):
                        pp = ps_big.tile([128, 384], f32, tag="big")
                        for dc in range(DC):
                            nc.tensor.matmul(
                                pp[:], xn[:, dc, 128 * tc4:128 * (tc4 + 1)],
                                wv[:, dc, :], start=(dc == 0),
                                stop=(dc == DC - 1))
                        nc.vector.tensor_copy(
                            vv[:, tc4, 6 * nv:6 * (nv + 1), 0:HD],
                            pp[:].rearrange("p (h w) -> p h w", h=6))

                # ---- own-chunk attention (no collective dependency)
                def sc_exp(kk_ap, p2, s, hp, mask_t=None):
                    s2 = ps_big.tile([128, 2, TOK], f32, tag="s2")
                    for h01 in range(2):
                        nc.tensor.matmul(
                            s2[:, h01, :], kk_ap[64 * h01:64 * h01 + 64, :],
                            q_sb[64 * h01:64 * h01 + 64, hp, :],
                            start=True, stop=True)
                    nc.scalar.activation(p2[:], s2[:], EXP, scale=0.125,
                                         bias=btab_sb[:, s:s + 1])
                    if mask_t is not None:
                        for h01 in range(2):
                            nc.vector.tensor_mul(p2[:, h01, :], p2[:, h01, :],
                                                 mrel_sb[:, mask_t, :])

                if ABLATE == "attn":
                    nc.vector.memset(o_sb[:], 0.001)
                    nc.vector.memset(o_part[:], 0.001)
                else:
                    for hp in range(DC):
                        o_psA = ps_att.tile([HD + 1, TOK], f32, tag="att",
                                            name=f"oownA{l}_{hp}")
                        o_psB = ps_att.tile([HD + 1, TOK], f32, tag="att",
                                            name=f"oownB{l}_{hp}")
                        o_pss = (o_psA, o_psB)
                        for t in range(4):
                            p2 = pbuf.tile([128, 2, TOK], bf16, tag="p")
                            sc_exp(k_own[:, hp, 128 * t:128 * (t + 1)],
                                   p2[:], t, hp, mask_t=t)
                            for h01 in range(2):
                                nc.tensor.matmul(
                                    o_pss[h01][:],
                                    vv[:, t, 2 * hp + h01, :], p2[:, h01, :],
                                    start=(t == 0), stop=(t == 3))
                        for h01 in range(2):
                            nc.vector.tensor_copy(o_part[:, hp, h01, :],
                                                  o_pss[h01][:])

                # ---- gather peer xn (after AG), first chunk first
                for i, (o0, w) in enumerate(XSPLIT):
                    for dc in range(DC):
                        for pj in range(3):
                            nc.gpsimd.indirect_dma_start(
                                out=xg[:, dc, pj, o0:o0 + w],
                                out_offset=None, in_=xg_out[i][:],
                                in_offset=bass.IndirectOffsetOnAxis(
                                    ap=gofs_sb[:, 3 * dc + pj:
                                               3 * dc + pj + 1], axis=0))

                PH_TCJ = [[0, 1], [2, 3]]
                for half in range(2):
                    # peer V for this phase's token chunks
                    for nv in range(2):
                        col0 = 2 * D + 384 * nv
                        wv = wcol_chunk(wqkv_d[l][:, col0:col0 + 384], n=384)
                        for pj in range(3):
                            for tcj in PH_TCJ[half]:
                                sv = 4 * pj + tcj
                                pp = ps_big.tile([128, 384], f32, tag="big")
                                for dc in range(DC):
                                    nc.tensor.matmul(
                                        pp[:],
                                        xg[:, dc, pj,
                                           128 * tcj:128 * (tcj + 1)],
                                        wv[:, dc, :], start=(dc == 0),
                                        stop=(dc == DC - 1))
                                nc.vector.tensor_copy(
                                    vv[:, 4 + sv, 6 * nv:6 * (nv + 1), 0:HD],
                                    pp[:].rearrange("p (h w) -> p h w", h=6))

                    if ABLATE == "attn":
                        continue
                    o0, w = XSPLIT[half]

                    def kk_groups(hp, kk):
                        """3 thunks, each projecting one peer's K chunk."""
                        def mk(pj):
                            def emit():
                                pp = ps_big.tile(
                                    [128, 256], f32, tag="big",
                                    name=f"kp{l}_{half}_{hp}_{pj}")
                                for dc in range(DC):
                                    nc.tensor.matmul(
                                        pp[:, 0:w],
                                        wk_all[:, dc,
                                               128 * hp:128 * (hp + 1)],
                                        xg[:, dc, pj, o0:o0 + w],
                                        start=(dc == 0),
                                        stop=(dc == DC - 1))
                                nc.vector.tensor_copy(kk[:, pj, 0:w],
                                                      pp[:, 0:w])
                            return emit
                        return [mk(pj) for pj in range(3)]

                    kk_cur = kkp.tile([128, 3, 256], bf16, tag="kk",
                                      name=f"kk{l}_{half}_0")
                    for g in kk_groups(0, kk_cur):
                        g()
                    for hp in range(DC):
                        # peer K of the next head pair, interleaved into the
                        # exp-paced gaps of this head pair's score loop
                        if hp < DC - 1:
                            kk_next = kkp.tile([128, 3, 256], bf16,
                                               tag="kk",
                                               name=f"kk{l}_{half}_{hp+1}")
                            nxt = kk_groups(hp + 1, kk_next)
                        else:
                            kk_next, nxt = None, []
                        o_psA = ps_att.tile([HD + 1, TOK], f32, tag="att",
                                            name=f"opA{l}_{half}_{hp}")
                        o_psB = ps_att.tile([HD + 1, TOK], f32, tag="att",
                                            name=f"opB{l}_{half}_{hp}")
                        o_pss = (o_psA, o_psB)
                        ntc = len(PH_TCJ[half])
                        for si in range(3 * ntc):
                            pj, t2 = si // ntc, si % ntc
                            tcj = PH_TCJ[half][t2]
                            sv = 4 * pj + tcj
                            p2 = pbuf.tile([128, 2, TOK], bf16, tag="p")
                            sc_exp(kk_cur[:, pj, 128 * t2:128 * (t2 + 1)],
                                   p2[:], 4 + sv, hp)
                            if si % 2 == 0 and si // 2 < len(nxt):
                                nxt[si // 2]()
                            for h01 in range(2):
                                nc.tensor.matmul(
                                    o_pss[h01][:],
                                    vv[:, 4 + sv, 2 * hp + h01, :],
                                    p2[:, h01, :],
                                    start=(si == 0),
                                    stop=(si == 3 * ntc - 1))
                        if half == 0:
                            for h01 in range(2):
                                nc.vector.tensor_add(
                                    o_part[:, hp, h01, :],
                                    o_pss[h01][:], o_part[:, hp, h01, :])
                            kk_cur = kk_next
                        else:
                            for h01 in range(2):
                                osum = tmp.tile([HD + 1, TOK], f32,
                                                tag="osum")
                                nc.vector.tensor_add(
                                    osum[:], o_pss[h01][:],
                                    o_part[:, hp, h01, :])
                                rrow = rows.tile([1, TOK], f32, tag="rrow",
                                                 bufs=2)
                                nc.vector.reciprocal(rrow[:],
                                                     osum[HD:HD + 1, :])
                                nb_ps = ps_big.tile([64, TOK], f32,
                                                    tag="big")
                                nc.tensor.matmul(nb_ps[:],
                                                 ones_row[:, 0:64],
                                                 rrow[:], start=True,
                                                 stop=True)
                                nb_sb = tmp.tile([64, TOK], bf16, tag="nb")
                                nc.vector.tensor_copy(nb_sb[:], nb_ps[:])
                                nc.vector.tensor_mul(
                                    o_sb[64 * h01:64 * h01 + 64, hp, :],
                                    osum[0:HD, :], nb_sb[:])
                        kk_cur = kk_next

                # ---- out projection + residual (LN2 stats interleaved)
                st2 = ln_new()
                for oc in range(DC):
                    woc = wstream.tile([128, DC, 128], bf16, tag="wk",
                                       name=f"woc{l}_{oc}", bufs=2)
                    nc.sync.dma_start(
                        woc[:], wout_d[l][:, 128 * oc:128 * (oc + 1)]
                        .rearrange("(c p) n -> p c n", p=128))
                    pp = ps_big.tile([128, TOK], f32, tag="big")
                    for dc in range(DC):
                        nc.tensor.matmul(
                            pp[:], woc[:, dc, :], o_sb[:, dc, :],
                            start=(dc == 0), stop=(dc == DC - 1))
                    nc.vector.tensor_add(x_fm[:, oc, :], pp[:], x_fm[:, oc, :])
                    ln_feed(st2, oc)

                # ---- LN2 + FFN up + silu (silu straight from PSUM)
                ln_finish(st2, xn)
                for ocp in range(DFC // 2):
                    wc = wcol_chunk(wup_d[l][:, 256 * ocp:256 * (ocp + 1)],
                                    n=256)
                    for k2 in range(2):
                        oc = 2 * ocp + k2
                        sl = slice(128 * k2, 128 * (k2 + 1))
                        pp = ps_big.tile([128, TOK], f32, tag="big")
                        for dc in range(DC):
                            nc.tensor.matmul(pp[:], wc[:, dc, sl],
                                             xn[:, dc, :],
                                             start=(dc == 0),
                                             stop=(dc == DC - 1))
                        nc.scalar.activation(s_sb[:, oc, :], pp[:], SILU)

                # ---- FFN down + residual (single pass, 6 accumulators)
                s2a = ps_big.tile([128, 2, TOK], f32, tag="s2",
                                  name=f"dn_s2a_{l}")
                s2b = ps_big.tile([128, 2, TOK], f32, tag="s2",
                                  name=f"dn_s2b_{l}")
                pb0 = ps_big.tile([128, TOK], f32, tag="big",
                                  name=f"dn_pb0_{l}")
                pb1 = ps_big.tile([128, TOK], f32, tag="big",
                                  name=f"dn_pb1_{l}")
                accs = [s2a[:, 0, :], s2a[:, 1, :], s2b[:, 0, :],
                        s2b[:, 1, :], pb0[:], pb1[:]]
                for dfc in range(DFC):
                    wd_sb = wstream.tile([128, D], bf16, tag="wdn",
                                         name=f"wd{l}_{dfc}")
                    nc.sync.dma_start(wd_sb[:], wdn_d[l, 128 * dfc:
                                                      128 * (dfc + 1), :])
                    for oc in range(DC):
                        nc.tensor.matmul(
                            accs[oc], wd_sb[:, 128 * oc:128 * (oc + 1)],
                            s_sb[:, dfc, :], start=(dfc == 0),
                            stop=(dfc == DFC - 1))
                stn = ln_new()
                for oc in range(DC):
                    nc.vector.tensor_add(x_fm[:, oc, :], accs[oc],
                                         x_fm[:, oc, :])
                    ln_feed(stn, oc)
                _STATS.append(stn)

            # ---------------- final LN -> xf; 2-half AllGather ----
            def _ship_xh(i):
                nc.sync.dma_start(
                    xh_in[i][:].rearrange("(c p) f -> p c f", p=128),
                    xf_sb[:, :, 256 * i:256 * (i + 1)])
                nc.gpsimd.collective_compute(
                    "AllGather", mybir.AluOpType.bypass,
                    replica_groups=GROUPS4, ins=[xh_in[i][:]],
                    outs=[xh_out[i][:]])

            ln_finish(_STATS.pop(), xf_sb, ship=_ship_xh)

          # ---------------- head phase (separate pool scope) --------------
          with contextlib.ExitStack() as ctx:
            hw = ctx.enter_context(tc.tile_pool(name="hw", bufs=1))
            lg = ctx.enter_context(tc.tile_pool(name="lg", bufs=4))
            ps_big2 = ctx.enter_context(
                tc.tile_pool(name="ps_big2", bufs=3, space="PSUM"))

            # resident vocab-shard embedding (transposed), 16 chunk loads
            teT_sb = hw.tile([128, DC, VPAD], bf16, tag="teT")
            for vc in range(VPAD // 512):
                nc.sync.dma_start(
                    teT_sb[:, :, 512 * vc:512 * (vc + 1)],
                    teT_d[:, 512 * vc:512 * (vc + 1)]
                    .rearrange("(c p) n -> p c n", p=128))

            # peer hidden states, gathered per (half, dc)
            xa = hw.tile([128, DC, 2, 3, 256], bf16, tag="xa")
            for half in range(2):
                for dc in range(DC):
                    for pj in range(3):
                        nc.gpsimd.indirect_dma_start(
                            out=xa[:, dc, half, pj, :],
                            out_offset=None, in_=xh_out[half][:],
                            in_offset=bass.IndirectOffsetOnAxis(
                                ap=gofs_sb[:, 3 * dc + pj:3 * dc + pj + 1],
                                axis=0))

            def head_block(sl, lhsT_fn, ti):
                for vc in range(VPAD // 512):
                    pp = ps_big2.tile([128, 512], f32, tag="big2")
                    for dc in range(DC):
                        nc.tensor.matmul(
                            pp[:], lhsT_fn(dc),
                            teT_sb[:, dc, 512 * vc:512 * (vc + 1)],
                            start=(dc == 0), stop=(dc == DC - 1))
                    lg_sb = lg.tile([128, 512], f32, tag="lg")
                    if (ti + vc) % 2 == 0:
                        nc.vector.tensor_copy(lg_sb[:], pp[:])
                    else:
                        nc.scalar.copy(lg_sb[:], pp[:])
                    nc.sync.dma_start(
                        out_d[128 * sl:128 * (sl + 1),
                              512 * vc:512 * (vc + 1)],
                        lg_sb[:])

            if ABLATE != "head":
                for t in range(4):  # own tokens first
                    head_block(
                        t,
                        (lambda tt: (lambda dc:
                                     xf_sb[:, dc, 128 * tt:128 * (tt + 1)]))(t),
                        t)
                for half in range(2):
                    for s in range(6):
                        pj, tj = s // 2, s % 2
                        head_block(
                            4 + 6 * half + s,
                            (lambda hh, pp_, tt: (lambda dc:
                             xa[:, dc, hh, pp_,
                                128 * tt:128 * (tt + 1)]))(half, pj, tj),
                            s)

    nc.compile()
    return nc


def _make_runner(nc):
    import jax
    import jax.numpy as jnp
    from jax.sharding import Mesh, PartitionSpec, NamedSharding
    from jax.experimental.shard_map import shard_map
    from concourse import bass2jax, mybir

    bass2jax.install_neuronx_cc_hook()
    partition_name = (nc.partition_id_tensor.name
                      if nc.partition_id_tensor else None)

    SHARED = {"te", "wqkv", "wout", "wup", "wdn"}
    in_names, out_names, out_avals = [], [], []
    for alloc in nc.m.functions[0].allocations:
        if not isinstance(alloc, mybir.MemoryLocationSet):
            continue
        name = alloc.memorylocations[0].name
        if alloc.kind == "ExternalInput":
            if name != partition_name:
                in_names.append(name)
        elif alloc.kind == "ExternalOutput":
            out_names.append(name)
            out_avals.append(jax.core.ShapedArray(
                tuple(alloc.tensor_shape), mybir.dt.np(alloc.dtype)))
    n_params = len(in_names)
    full_names = list(in_names) + list(out_names)
    if partition_name is not None:
        full_names.append(partition_name)

    def _body(*args):
        operands = list(args)
        if partition_name is not None:
            operands.append(bass2jax.partition_id_tensor())
        outs = bass2jax._bass_exec_p.bind(
            *operands,
            out_avals=tuple(out_avals),
            in_names=tuple(full_names),
            out_names=tuple(out_names),
            lowering_input_output_aliases=(),
            sim_require_finite=True,
            sim_require_nnan=True,
            nc=nc,
        )
        return tuple(outs)

    devices = jax.devices()[:NCORES]
    mesh = Mesh(np.asarray(devices), ("core",))
    in_specs = tuple(
        PartitionSpec() if n in SHARED else PartitionSpec("core")
        for n in in_names) + (PartitionSpec("core"),) * len(out_names)
    out_specs = (PartitionSpec("core"),) * len(out_names)
    donate = tuple(range(n_params, n_params + len(out_names)))
    sharded = jax.jit(
        shard_map(_body, mesh=mesh, in_specs=in_specs, out_specs=out_specs,
                  check_rep=False),
        donate_argnums=donate, keep_unused=True)

    sharded_nodonate = jax.jit(
        shard_map(_body, mesh=mesh, in_specs=in_specs, out_specs=out_specs,
                  check_rep=False),
        keep_unused=True)

    shard8 = NamedSharding(mesh, PartitionSpec("core"))
    repl = NamedSharding(mesh, PartitionSpec())

    zfns = [
        jax.jit(
            (lambda av: (lambda: jnp.zeros((NCORES * av.shape[0],)
                                           + av.shape[1:], av.dtype)))(av),
            out_shardings=shard8)
        for av in out_avals
    ]

    def put_inputs(per_core_maps, shared_map):
        dev = []
        for n in in_names:
            if n in SHARED:
                dev.append(jax.device_put(shared_map[n], repl))
            else:
                arr = np.concatenate([m[n] for m in per_core_maps], axis=0)
                dev.append(jax.device_put(arr, shard8))
        return dev

    def run(dev_inputs):
        zeros = [zf() for zf in zfns]
        outs = sharded(*dev_inputs, *zeros)
        jax.block_until_ready(outs)
        return {n: outs[i] for i, n in enumerate(out_names)}

    def run_burst(dev_inputs, n):
        zeros = [zf() for zf in zfns]
        jax.block_until_ready(zeros)
        t0 = time.time()
        outs = None
        for _ in range(n):
            outs = sharded_nodonate(*dev_inputs, *zeros)
        jax.block_until_ready(outs)
        return time.time() - t0

    return put_inputs, run, run_burst


def _prepare_inputs(ids, te, pe):
    bf = ml_dtypes.bfloat16
    ids = np.asarray(ids)
    te_f = np.asarray(te, dtype=np.float32)
    per_core = []
    for c in range(NCORES):
        b, cc = c // 4, c % 4
        peers = [r for r in range(4) if r != cc]
        sl = slice(TOK * cc, TOK * (cc + 1))
        idx = ids[b, sl].astype(np.int32).reshape(TOK, 1)
        pe_s = np.asarray(pe[sl], dtype=np.float32)
        # relative diagonal masks: mrel[t][i, j] = 1 if 128*t + i <= j
        ki = (128 * np.arange(4)[:, None, None]
              + np.arange(128)[None, :, None])
        qj = np.arange(TOK)[None, None, :]
        mrel = (ki <= qj).astype(bf)
        # exp bias: own slots 0, peer slot visible iff peer rank < cc
        btab = np.zeros((128, NKT), dtype=np.float32)
        for s in range(12):
            if peers[s // 4] >= cc:
                btab[:, 4 + s] = -30.0
        # gather offsets: row = 768*peer + 128*dc + p
        gofs = np.zeros((128, DC * 3), dtype=np.int32)
        for dc in range(DC):
            for j in range(3):
                gofs[:, 3 * dc + j] = (768 * peers[j] + 128 * dc
                                       + np.arange(128))
        teT_s = np.zeros((D, VPAD), dtype=bf)
        teT_s[:, :VSH] = te_f[VSH * cc:VSH * (cc + 1), :].T.astype(bf)
        if os.environ.get("KERNEL_GOFS0"):
            gofs[:] = 0
        if os.environ.get("KERNEL_BTAB0"):
            btab[:] = 0.0
        if os.environ.get("KERNEL_MREL1"):
            mrel[:] = 1.0
        per_core.append({"idx": idx, "pe_s": pe_s, "mrel": mrel,
                         "btab": btab, "gofs": gofs, "teT_s": teT_s})
    shared_map = {
        "te": te_f,
        "wqkv": _STATE["shared"]["wqkv"],
        "wout": _STATE["shared"]["wout"],
        "wup": _STATE["shared"]["wup"],
        "wdn": _STATE["shared"]["wdn"],
    }
    return per_core, shared_map


def kernel(ids, te, pe, ln1_s, ln1_b, qkv_w, qkv_b, out_w, out_b,
           ln2_s, ln2_b, up_w, up_b, dn_w, dn_b, lnf_s, lnf_b):
    bf = ml_dtypes.bfloat16
    # identity LN affine params and zero biases (true for this model family)
    for z in (ln1_b, ln2_b, lnf_b, qkv_b, out_b, up_b, dn_b):
        assert not np.asarray(z).any(), "nonzero bias unsupported"
    for o in (ln1_s, ln2_s, lnf_s):
        assert np.all(np.asarray(o) == 1.0), "non-identity LN scale unsupported"

    if "run" not in _STATE:
        _STATE["shared"] = {
            "wqkv": np.ascontiguousarray(np.asarray(qkv_w)).astype(bf),
            "wout": np.ascontiguousarray(np.asarray(out_w)).astype(bf),
            "wup": np.ascontiguousarray(np.asarray(up_w)).astype(bf),
            "wdn": np.ascontiguousarray(np.asarray(dn_w)).astype(bf),
        }
        nc = _build_program()
        put_inputs, run, run_burst = _make_runner(nc)
        _STATE["put_inputs"] = put_inputs
        _STATE["run"] = run
        _STATE["run_burst"] = run_burst

    per_core, shared_map = _prepare_inputs(ids, te, pe)
    dev_inputs = _STATE["put_inputs"](per_core, shared_map)
    _STATE["dev_inputs"] = dev_inputs
    outs = _STATE["run"](dev_inputs)
    logits = np.asarray(outs["logits"])  # [8*2048, 8192]
    logits = logits.reshape(NCORES, GTOK, VPAD)[:, :, :VSH]
    # core c = 4b + cc: rows are slot-ordered; map slots back to tokens
    full = np.empty((B, T, V), dtype=np.float32)
    for c in range(NCORES):
        b, cc = c // 4, c % 4
        peers = [r for r in range(4) if r != cc]
        tokmap = np.empty(GTOK, dtype=np.int64)
        for t in range(4):  # own slots
            tokmap[128 * t:128 * (t + 1)] = (TOK * cc + 128 * t
                                             + np.arange(128))
        for half in range(2):
            for s in range(6):
                pj, tj = s // 2, s % 2
                sl_ = 4 + 6 * half + s
                base = TOK * peers[pj] + 256 * half + 128 * tj
                tokmap[128 * sl_:128 * (sl_ + 1)] = base + np.arange(128)
        full[b, tokmap, VSH * cc:VSH * (cc + 1)] = logits[c]
    return full


# revision 27
# speedup vs baseline: 1.5397x; 1.0496x over previous
"""Bass/Tile TRN2 kernel for a 4-layer dense transformer (D=768, H=12, DF=3072,
V=32000, B=2, T=2048) sharded across 8 NeuronCores.

Sharding: each core owns 512 tokens (core c -> batch c//4, tokens 512*(c%4)...).
Per layer the LN1-normalized hidden states are AllGathered across the 4-core
batch group (split into two token-half collectives so compute overlaps), and
every core computes K/V for all 2048 context tokens locally -- one small
collective per layer instead of shipping K and V.

Key chunks are processed in a per-core slot order: slots 0-3 are the core's
own (causally diagonal) chunks, computable before any collective; slots 4-15
are peer chunks.  Causal masking uses static relative masks on the diagonal
slots plus a per-core additive bias table on the exp (fully-masked chunks get
-1e4 so exp underflows to zero) -- no elementwise mask is needed off-diagonal.

For the tied LM head the final hidden states are AllGathered across the batch
group in two halves; the vocabulary is sharded V/4=8000 (padded 8192) per
core.  Head output rows are written in slot order (own tokens first, enabling
compute during the gather) and reordered on the host.

Everything numerical is bf16/f32 (fp8 attention was measured to breach the
2e-2 tolerance).  Layernorms are materialized explicitly (normalized copies),
so no projection corrections are needed anywhere.
"""

import os
import sys
import time

for _p in ("/opt/trn_rl_repo", "/root/.axon_site/_ro/trn_rl_repo"):
    if os.path.isdir(_p) and _p not in sys.path:
        sys.path.insert(0, _p)

import numpy as np
import ml_dtypes

D, DF, H, L, V, T_MAX = 768, 3072, 12, 4, 32000, 2048
HD = D // H          # 64
B, T = 2, 2048
NCORES = 8
TOK = 512            # tokens per core
GTOK = 4 * TOK       # tokens per batch group
DC = D // 128        # 6 feature chunks
DFC = DF // 128      # 24
VSH = V // 4         # 8000 vocab per core (sharded within batch group)
VPAD = 8192          # padded vocab shard
NKT = 16             # key chunks of 128 (full 2048 context)
EPS = 1e-5

_STATE = {}
ABLATE = os.environ.get("KERNEL_ABLATE", "")


def _build_program():
    import concourse.bass as bass
    import concourse.tile as tile
    from concourse import bacc, mybir
    from concourse.masks import make_identity

    f32 = mybir.dt.float32
    bf16 = mybir.dt.bfloat16
    i32 = mybir.dt.int32
    EXP = mybir.ActivationFunctionType.Exp
    SILU = mybir.ActivationFunctionType.Silu
    SQRT = mybir.ActivationFunctionType.Sqrt

    nc = bacc.Bacc("TRN2", target_bir_lowering=False, debug=False,
                   num_devices=NCORES)

    # ---------------- DRAM I/O ----------------
    te_d = nc.dram_tensor("te", [V, D], f32, kind="ExternalInput")
    wqkv_d = nc.dram_tensor("wqkv", [L, D, 3 * D], bf16, kind="ExternalInput")
    wout_d = nc.dram_tensor("wout", [L, D, D], bf16, kind="ExternalInput")
    wup_d = nc.dram_tensor("wup", [L, D, DF], bf16, kind="ExternalInput")
    wdn_d = nc.dram_tensor("wdn", [L, DF, D], bf16, kind="ExternalInput")
    # per-core
    idx_d = nc.dram_tensor("idx", [TOK, 1], i32, kind="ExternalInput")
    pe_d = nc.dram_tensor("pe_s", [TOK, D], f32, kind="ExternalInput")
    mrel_d = nc.dram_tensor("mrel", [4, 128, TOK], bf16, kind="ExternalInput")
    btab_d = nc.dram_tensor("btab", [128, NKT], f32, kind="ExternalInput")
    gofs_d = nc.dram_tensor("gofs", [128, DC * 3], i32, kind="ExternalInput")
    teT_d = nc.dram_tensor("teT_s", [D, VPAD], bf16, kind="ExternalInput")
    # output
    out_d = nc.dram_tensor("logits", [GTOK, VPAD], f32, kind="ExternalOutput")

    # internal DRAM for collectives (xn halves per layer, xf halves at end)
    XSPLIT = [(0, 256), (256, 256)]
    xg_in = [nc.dram_tensor(f"xg{i}_in", [D, w], bf16)
             for i, (_, w) in enumerate(XSPLIT)]
    xg_out = [nc.dram_tensor(f"xg{i}_out", [4 * D, w], bf16)
              for i, (_, w) in enumerate(XSPLIT)]
    xh_in = [nc.dram_tensor(f"xh{i}_in", [D, TOK // 2], bf16)
             for i in range(2)]
    xh_out = [nc.dram_tensor(f"xh{i}_out", [4 * D, TOK // 2], bf16)
              for i in range(2)]

    GROUPS4 = [[0, 1, 2, 3], [4, 5, 6, 7]]

    with tile.TileContext(nc) as tc:
        import contextlib
        with tc.tile_pool(name="xfp", bufs=1) as xfp, \
                tc.tile_pool(name="cstp", bufs=1) as cstp:
          with contextlib.ExitStack() as ctx:
            # ---------------- pools ----------------
            const = ctx.enter_context(tc.tile_pool(name="const", bufs=1))
            xp = ctx.enter_context(tc.tile_pool(name="xp", bufs=1))
            act = ctx.enter_context(tc.tile_pool(name="act", bufs=1))
            wstream = ctx.enter_context(tc.tile_pool(name="wstream", bufs=3))
            rows = ctx.enter_context(tc.tile_pool(name="rows", bufs=1))
            tmp = ctx.enter_context(tc.tile_pool(name="tmp", bufs=2))
            pbuf = ctx.enter_context(tc.tile_pool(name="pbuf", bufs=4))
            kkp = ctx.enter_context(tc.tile_pool(name="kkp", bufs=2))
            ps_big = ctx.enter_context(
                tc.tile_pool(name="ps_big", bufs=2, space="PSUM"))
            ps_att = ctx.enter_context(
                tc.tile_pool(name="ps_att", bufs=2, space="PSUM"))
            ps_row = ps_att

            # ---------------- constants ----------------
            ones_col = const.tile([128, 1], f32, tag="ones_col")
            nc.vector.memset(ones_col[:], 1.0)
            ones_row = const.tile([1, 128], f32, tag="ones_row")
            nc.vector.memset(ones_row[:], 1.0)
            eps_t = const.tile([1, 1], f32, tag="eps")
            nc.vector.memset(eps_t[:], EPS)
            ident = const.tile([128, 128], f32, tag="ident")
            make_identity(nc, ident[:])
            mrel_sb = const.tile([128, 4, TOK], bf16, tag="mrel")
            nc.sync.dma_start(mrel_sb[:], mrel_d[:].rearrange("t p f -> p t f"))
            btab_sb = const.tile([128, NKT], f32, tag="btab")
            nc.sync.dma_start(btab_sb[:], btab_d[:])
            gofs_sb = cstp.tile([128, DC * 3], i32, tag="gofs")
            nc.sync.dma_start(gofs_sb[:], gofs_d[:])

            # persistent activations
            x_fm = xp.tile([128, DC, TOK], f32, tag="x_fm")
            xf_sb = xfp.tile([128, DC, TOK], bf16, tag="xf")

            # ---------------- embedding ----------------
            idx_sb = tmp.tile([128, 4, 1], i32, tag="idx")
            nc.sync.dma_start(
                idx_sb[:], idx_d[:].rearrange("(tc p) o -> p tc o", p=128))
            emb_ts = []
            for tc4 in range(4):
                emb_t = tmp.tile([128, D], f32, tag="emb", bufs=3,
                                 name=f"emb{tc4}")
                nc.gpsimd.indirect_dma_start(
                    out=emb_t[:], out_offset=None, in_=te_d[:],
                    in_offset=bass.IndirectOffsetOnAxis(
                        ap=idx_sb[:, tc4, 0:1], axis=0))
                emb_ts.append(emb_t)
            for tc4 in range(4):
                emb_t = emb_ts[tc4]
                pe_t = tmp.tile([128, D], f32, tag="pe", bufs=1)
                nc.sync.dma_start(pe_t[:], pe_d[128 * tc4:128 * (tc4 + 1), :])
                nc.vector.tensor_add(emb_t[:], emb_t[:], pe_t[:])
                for dc in range(DC):
                    tp = ps_att.tile([128, 128], f32, tag="att")
                    nc.tensor.transpose(
                        tp[:], emb_t[:, 128 * dc:128 * (dc + 1)], ident[:])
                    nc.vector.tensor_copy(
                        x_fm[:, dc, 128 * tc4:128 * (tc4 + 1)], tp[:])

            # ---------------- helpers ----------------
            def ln_new():
                sum_ps = ps_row.tile([1, TOK], f32, tag="att")
                sq_ps = ps_row.tile([1, TOK], f32, tag="att")
                return sum_ps, sq_ps

            def ln_feed(st, dc):
                sum_ps, sq_ps = st
                nc.tensor.matmul(sum_ps[:], ones_col[:], x_fm[:, dc, :],
                                 start=(dc == 0), stop=(dc == DC - 1))
                xsq = tmp.tile([128, TOK], f32, tag="xsq", bufs=1)
                nc.vector.tensor_mul(xsq[:], x_fm[:, dc, :], x_fm[:, dc, :])
                nc.tensor.matmul(sq_ps[:], ones_col[:], xsq[:],
                                 start=(dc == 0), stop=(dc == DC - 1))

            def ln_finish(st, dst, ship=None):
                sum_ps, sq_ps = st
                mu_row = rows.tile([1, TOK], f32, tag="mu")
                nc.vector.tensor_scalar_mul(mu_row[:], sum_ps[:], 1.0 / D)
                ex2 = rows.tile([1, TOK], f32, tag="ex2")
                nc.vector.tensor_scalar_mul(ex2[:], sq_ps[:], 1.0 / D)
                var = rows.tile([1, TOK], f32, tag="var")
                nc.vector.tensor_mul(var[:], mu_row[:], mu_row[:])
                nc.vector.tensor_sub(var[:], ex2[:], var[:])
                std = rows.tile([1, TOK], f32, tag="std")
                nc.scalar.activation(std[:], var[:], SQRT, bias=eps_t[:])
                rstd_row = rows.tile([1, TOK], f32, tag="rstd")
                nc.vector.reciprocal(rstd_row[:], std[:])
                bc_ps = ps_big.tile([128, TOK], f32, tag="big")
                nc.tensor.matmul(bc_ps[:], ones_row[:], rstd_row[:],
                                 start=True, stop=True)
                rstd_bc = rows.tile([128, TOK], f32, tag="rstd_bc")
                nc.vector.tensor_copy(rstd_bc[:], bc_ps[:])
                mb_ps = ps_big.tile([128, TOK], f32, tag="big")
                nc.tensor.matmul(mb_ps[:], ones_row[:], mu_row[:],
                                 start=True, stop=True)
                mu_bc = rows.tile([128, TOK], f32, tag="mu_bc")
                nc.vector.tensor_copy(mu_bc[:], mb_ps[:])
                for i in range(2):
                    cs = slice(256 * i, 256 * (i + 1))
                    for dc in range(DC):
                        xc = tmp.tile([128, 256], f32, tag="xsq", bufs=1)
                        nc.vector.tensor_sub(xc[:], x_fm[:, dc, cs],
                                             mu_bc[:, cs])
                        nc.vector.tensor_mul(dst[:, dc, cs], xc[:],
                                             rstd_bc[:, cs])
                    if ship is not None:
                        ship(i)

            _uid = [0]
            _STATS = []

            def wcol_chunk(src_ap, n):
                wc = wstream.tile([128, DC, n], bf16, tag="wchunk",
                                  name=f"wc{_uid[0]}")
                _uid[0] += 1
                nc.sync.dma_start(
                    wc[:], src_ap.rearrange("(c p) n -> p c n", p=128))
                return wc

            xn = act.tile([128, DC, TOK], bf16, tag="xn")
            xg = act.tile([128, DC, 3, TOK], bf16, tag="xg")
            q_sb = act.tile([128, DC, TOK], bf16, tag="q")
            k_own = act.tile([128, DC, TOK], bf16, tag="k_own")
            vv = act.tile([128, NKT, H, HD + 1], bf16, tag="vv")
            o_sb = act.tile([128, DC, TOK], bf16, tag="o")
            o_part = act.tile([HD + 1, DC, 2, TOK], bf16, tag="o_part")
            s_sb = act.tile([128, DFC, TOK], bf16, tag="s_silu")

            # ---------------- layers ----------------
            for l in range(L):
                # ---- LN1 -> xn; ship each half as soon as it is ready
                def _ship_xg(i):
                    o0, w = XSPLIT[i]
                    nc.sync.dma_start(
                        xg_in[i][:].rearrange("(c p) f -> p c f", p=128),
                        xn[:, :, o0:o0 + w])
                    nc.gpsimd.collective_compute(
                        "AllGather", mybir.AluOpType.bypass,
                        replica_groups=GROUPS4, ins=[xg_in[i][:]],
                        outs=[xg_out[i][:]])

                if l == 0:
                    st1 = ln_new()
                    for dc in range(DC):
                        ln_feed(st1, dc)
                else:
                    st1 = _STATS.pop()
                ln_finish(st1, xn, ship=_ship_xg)

                # ---- own projections (overlap the gathers)
                def proj_own(dst, base):
                    for ocp in range(3):
                        col0 = base + 256 * ocp
                        wc = wcol_chunk(wqkv_d[l][:, col0:col0 + 256], n=256)
                        for k2 in range(2):
                            sl = slice(128 * k2, 128 * (k2 + 1))
                            pp = ps_big.tile([128, TOK], f32, tag="big")
                            for dc in range(DC):
                                nc.tensor.matmul(pp[:], wc[:, dc, sl],
                                                 xn[:, dc, :],
                                                 start=(dc == 0),
                                                 stop=(dc == DC - 1))
                            nc.vector.tensor_copy(dst[:, 2 * ocp + k2, :],
                                                  pp[:])

                wk_all = wstream.tile([128, DC, D], bf16, tag="wkall",
                                      name=f"wka{l}", bufs=1)
                nc.sync.dma_start(
                    wk_all[:], wqkv_d[l][:, D:2 * D]
                    .rearrange("(c p) n -> p c n", p=128))
                proj_own(q_sb, 0)
                for oc in range(DC):
                    pp = ps_big.tile([128, TOK], f32, tag="big",
                                     name=f"ko{l}_{oc}")
                    for dc in range(DC):
                        nc.tensor.matmul(
                            pp[:], wk_all[:, dc, 128 * oc:128 * (oc + 1)],
                            xn[:, dc, :], start=(dc == 0),
                            stop=(dc == DC - 1))
                    nc.vector.tensor_copy(k_own[:, oc, :], pp[:])

                nc.vector.memset(vv[:, :, :, HD:HD + 1], 1.0)
                for nv in range(2):
                    col0 = 2 * D + 384 * nv
                    wv = wcol_chunk(wqkv_d[l][:, col0:col0 + 384], n=384)
                    for tc4 in range(4):
                        pp = ps_big.tile([128, 384], f32, tag="big")
                        for dc in range(DC):
                            nc.tensor.matmul(
                                pp[:], xn[:, dc, 128 * tc4:128 * (tc4 + 1)],
                                wv[:, dc, :], start=(dc == 0),
                                stop=(dc == DC - 1))
                        nc.vector.tensor_copy(
                            vv[:, tc4, 6 * nv:6 * (nv + 1), 0:HD],
                            pp[:].rearrange("p (h w) -> p h w", h=6))

                # ---- own-chunk attention (no collective dependency)
                def sc_exp(kk_ap, p2, s, hp, mask_t=None):
                    s2 = ps_big.tile([128, 2, TOK], f32, tag="s2")
                    for h01 in range(2):
                        nc.tensor.matmul(
                            s2[:, h01, :], kk_ap[64 * h01:64 * h01 + 64, :],
                            q_sb[64 * h01:64 * h01 + 64, hp, :],
                            start=True, stop=True)
                    nc.scalar.activation(p2[:], s2[:], EXP, scale=0.125,
                                         bias=btab_sb[:, s:s + 1])
                    if mask_t is not None:
                        for h01 in range(2):
                            nc.vector.tensor_mul(p2[:, h01, :], p2[:, h01, :],
                                                 mrel_sb[:, mask_t, :])

                if ABLATE == "attn":
                    nc.vector.memset(o_sb[:], 0.001)
                    nc.vector.memset(o_part[:], 0.001)
                else:
                    for hp in range(DC):
                        o_psA = ps_att.tile([HD + 1, TOK], f32, tag="att",
                                            name=f"oownA{l}_{hp}")
                        o_psB = ps_att.tile([HD + 1, TOK], f32, tag="att",
                                            name=f"oownB{l}_{hp}")
                        o_pss = (o_psA, o_psB)
                        for t in range(4):
                            p2 = pbuf.tile([128, 2, TOK], bf16, tag="p")
                            sc_exp(k_own[:, hp, 128 * t:128 * (t + 1)],
                                   p2[:], t, hp, mask_t=t)
                            for h01 in range(2):
                                nc.tensor.matmul(
                                    o_pss[h01][:],
                                    vv[:, t, 2 * hp + h01, :], p2[:, h01, :],
                                    start=(t == 0), stop=(t == 3))
                        for h01 in range(2):
                            nc.vector.tensor_copy(o_part[:, hp, h01, :],
                                                  o_pss[h01][:])

                # ---- gather peer xn (after AG), first chunk first
                for i, (o0, w) in enumerate(XSPLIT):
                    for dc in range(DC):
                        for pj in range(3):
                            nc.gpsimd.indirect_dma_start(
                                out=xg[:, dc, pj, o0:o0 + w],
                                out_offset=None, in_=xg_out[i][:],
                                in_offset=bass.IndirectOffsetOnAxis(
                                    ap=gofs_sb[:, 3 * dc + pj:
                                               3 * dc + pj + 1], axis=0))

                PH_TCJ = [[0, 1], [2, 3]]
                for half in range(2):
                    # peer V for this phase's token chunks
                    for nv in range(2):
                        col0 = 2 * D + 384 * nv
                        wv = wcol_chunk(wqkv_d[l][:, col0:col0 + 384], n=384)
                        for pj in range(3):
                            for tcj in PH_TCJ[half]:
                                sv = 4 * pj + tcj
                                pp = ps_big.tile([128, 384], f32, tag="big")
                                for dc in range(DC):
                                    nc.tensor.matmul(
                                        pp[:],
                                        xg[:, dc, pj,
                                           128 * tcj:128 * (tcj + 1)],
                                        wv[:, dc, :], start=(dc == 0),
                                        stop=(dc == DC - 1))
                                nc.vector.tensor_copy(
                                    vv[:, 4 + sv, 6 * nv:6 * (nv + 1), 0:HD],
                                    pp[:].rearrange("p (h w) -> p h w", h=6))

                    if ABLATE == "attn":
                        continue
                    o0, w = XSPLIT[half]

                    def kk_groups(hp, kk):
                        """3 thunks, each projecting one peer's K chunk."""
                        def mk(pj):
                            def emit():
                                pp = ps_big.tile(
                                    [128, 256], f32, tag="big",
                                    name=f"kp{l}_{half}_{hp}_{pj}")
                                for dc in range(DC):
                                    nc.tensor.matmul(
                                        pp[:, 0:w],
                                        wk_all[:, dc,
                                               128 * hp:128 * (hp + 1)],
                                        xg[:, dc, pj, o0:o0 + w],
                                        start=(dc == 0),
                                        stop=(dc == DC - 1))
                                nc.vector.tensor_copy(kk[:, pj, 0:w],
                                                      pp[:, 0:w])
                            return emit
                        return [mk(pj) for pj in range(3)]

                    kk_cur = kkp.tile([128, 3, 256], bf16, tag="kk",
                                      name=f"kk{l}_{half}_0")
                    for g in kk_groups(0, kk_cur):
                        g()
                    for hp in range(DC):
                        # peer K of the next head pair, interleaved into the
                        # exp-paced gaps of this head pair's score loop
                        if hp < DC - 1:
                            kk_next = kkp.tile([128, 3, 256], bf16,
                                               tag="kk",
                                               name=f"kk{l}_{half}_{hp+1}")
                            nxt = kk_groups(hp + 1, kk_next)
                        else:
                            kk_next, nxt = None, []
                        o_psA = ps_att.tile([HD + 1, TOK], f32, tag="att",
                                            name=f"opA{l}_{half}_{hp}")
                        o_psB = ps_att.tile([HD + 1, TOK], f32, tag="att",
                                            name=f"opB{l}_{half}_{hp}")
                        o_pss = (o_psA, o_psB)
                        ntc = len(PH_TCJ[half])
                        for si in range(3 * ntc):
                            pj, t2 = si // ntc, si % ntc
                            tcj = PH_TCJ[half][t2]
                            sv = 4 * pj + tcj
                            p2 = pbuf.tile([128, 2, TOK], bf16, tag="p")
                            sc_exp(kk_cur[:, pj, 128 * t2:128 * (t2 + 1)],
                                   p2[:], 4 + sv, hp)
                            if si % 2 == 0 and si // 2 < len(nxt):
                                nxt[si // 2]()
                            for h01 in range(2):
                                nc.tensor.matmul(
                                    o_pss[h01][:],
                                    vv[:, 4 + sv, 2 * hp + h01, :],
                                    p2[:, h01, :],
                                    start=(si == 0),
                                    stop=(si == 3 * ntc - 1))
                        if half == 0:
                            for h01 in range(2):
                                nc.vector.tensor_add(
                                    o_part[:, hp, h01, :],
                                    o_pss[h01][:], o_part[:, hp, h01, :])
                            kk_cur = kk_next
                        else:
                            for h01 in range(2):
                                osum = tmp.tile([HD + 1, TOK], f32,
                                                tag="osum")
                                nc.vector.tensor_add(
                                    osum[:], o_pss[h01][:],
                                    o_part[:, hp, h01, :])
                                rrow = rows.tile([1, TOK], f32, tag="rrow",
                                                 bufs=2)
                                nc.vector.reciprocal(rrow[:],
                                                     osum[HD:HD + 1, :])
                                nb_ps = ps_big.tile([64, TOK], f32,
                                                    tag="big")
                                nc.tensor.matmul(nb_ps[:],
                                                 ones_row[:, 0:64],
                                                 rrow[:], start=True,
                                                 stop=True)
                                nb_sb = tmp.tile([64, TOK], bf16, tag="nb")
                                nc.vector.tensor_copy(nb_sb[:], nb_ps[:])
                                nc.vector.tensor_mul(
                                    o_sb[64 * h01:64 * h01 + 64, hp, :],
                                    osum[0:HD, :], nb_sb[:])
                        kk_cur = kk_next

                # ---- out projection + residual (LN2 stats interleaved)
                st2 = ln_new()
                for oc in range(DC):
                    woc = wstream.tile([128, DC, 128], bf16, tag="wk",
                                       name=f"woc{l}_{oc}", bufs=2)
                    nc.sync.dma_start(
                        woc[:], wout_d[l][:, 128 * oc:128 * (oc + 1)]
                        .rearrange("(c p) n -> p c n", p=128))
                    pp = ps_big.tile([128, TOK], f32, tag="big")
                    for dc in range(DC):
                        nc.tensor.matmul(
                            pp[:], woc[:, dc, :], o_sb[:, dc, :],
                            start=(dc == 0), stop=(dc == DC - 1))
                    nc.vector.tensor_add(x_fm[:, oc, :], pp[:], x_fm[:, oc, :])
                    ln_feed(st2, oc)

                # ---- LN2 + FFN up + silu (silu straight from PSUM)
                ln_finish(st2, xn)
                for ocp in range(DFC // 2):
                    wc = wcol_chunk(wup_d[l][:, 256 * ocp:256 * (ocp + 1)],
                                    n=256)
                    for k2 in range(2):
                        oc = 2 * ocp + k2
                        sl = slice(128 * k2, 128 * (k2 + 1))
                        pp = ps_big.tile([128, TOK], f32, tag="big")
                        for dc in range(DC):
                            nc.tensor.matmul(pp[:], wc[:, dc, sl],
                                             xn[:, dc, :],
                                             start=(dc == 0),
                                             stop=(dc == DC - 1))
                        nc.scalar.activation(s_sb[:, oc, :], pp[:], SILU)

                # ---- FFN down + residual (single pass, 6 accumulators)
                s2a = ps_big.tile([128, 2, TOK], f32, tag="s2",
                                  name=f"dn_s2a_{l}")
                s2b = ps_big.tile([128, 2, TOK], f32, tag="s2",
                                  name=f"dn_s2b_{l}")
                pb0 = ps_big.tile([128, TOK], f32, tag="big",
                                  name=f"dn_pb0_{l}")
                pb1 = ps_big.tile([128, TOK], f32, tag="big",
                                  name=f"dn_pb1_{l}")
                accs = [s2a[:, 0, :], s2a[:, 1, :], s2b[:, 0, :],
                        s2b[:, 1, :], pb0[:], pb1[:]]
                for dfc in range(DFC):
                    wd_sb = wstream.tile([128, D], bf16, tag="wdn",
                                         name=f"wd{l}_{dfc}", bufs=4)
                    nc.sync.dma_start(wd_sb[:], wdn_d[l, 128 * dfc:
                                                      128 * (dfc + 1), :])
                    for oc in range(DC):
                        nc.tensor.matmul(
                            accs[oc], wd_sb[:, 128 * oc:128 * (oc + 1)],
                            s_sb[:, dfc, :], start=(dfc == 0),
                            stop=(dfc == DFC - 1))
                stn = ln_new()
                for oc in range(DC):
                    nc.vector.tensor_add(x_fm[:, oc, :], accs[oc],
                                         x_fm[:, oc, :])
                    ln_feed(stn, oc)
                _STATS.append(stn)

            # ---------------- final LN -> xf; 2-half AllGather ----
            def _ship_xh(i):
                nc.sync.dma_start(
                    xh_in[i][:].rearrange("(c p) f -> p c f", p=128),
                    xf_sb[:, :, 256 * i:256 * (i + 1)])
                nc.gpsimd.collective_compute(
                    "AllGather", mybir.AluOpType.bypass,
                    replica_groups=GROUPS4, ins=[xh_in[i][:]],
                    outs=[xh_out[i][:]])

            ln_finish(_STATS.pop(), xf_sb, ship=_ship_xh)

          # ---------------- head phase (separate pool scope) --------------
          with contextlib.ExitStack() as ctx:
            hw = ctx.enter_context(tc.tile_pool(name="hw", bufs=1))
            lg = ctx.enter_context(tc.tile_pool(name="lg", bufs=4))
            ps_big2 = ctx.enter_context(
                tc.tile_pool(name="ps_big2", bufs=3, space="PSUM"))

            # resident vocab-shard embedding (transposed), 16 chunk loads
            teT_sb = hw.tile([128, DC, VPAD], bf16, tag="teT")
            for vc in range(VPAD // 512):
                nc.sync.dma_start(
                    teT_sb[:, :, 512 * vc:512 * (vc + 1)],
                    teT_d[:, 512 * vc:512 * (vc + 1)]
                    .rearrange("(c p) n -> p c n", p=128))

            # peer hidden states, gathered per (half, dc)
            xa = hw.tile([128, DC, 2, 3, 256], bf16, tag="xa")
            for half in range(2):
                for dc in range(DC):
                    for pj in range(3):
                        nc.gpsimd.indirect_dma_start(
                            out=xa[:, dc, half, pj, :],
                            out_offset=None, in_=xh_out[half][:],
                            in_offset=bass.IndirectOffsetOnAxis(
                                ap=gofs_sb[:, 3 * dc + pj:3 * dc + pj + 1],
                                axis=0))

            def head_block(sl, lhsT_fn, ti):
                for vc in range(VPAD // 512):
                    pp = ps_big2.tile([128, 512], f32, tag="big2")
                    for dc in range(DC):
                        nc.tensor.matmul(
                            pp[:], lhsT_fn(dc),
                            teT_sb[:, dc, 512 * vc:512 * (vc + 1)],
                            start=(dc == 0), stop=(dc == DC - 1))
                    lg_sb = lg.tile([128, 512], f32, tag="lg")
                    if (ti + vc) % 2 == 0:
                        nc.vector.tensor_copy(lg_sb[:], pp[:])
                    else:
                        nc.scalar.copy(lg_sb[:], pp[:])
                    nc.sync.dma_start(
                        out_d[128 * sl:128 * (sl + 1),
                              512 * vc:512 * (vc + 1)],
                        lg_sb[:])

            if ABLATE != "head":
                for t in range(4):  # own tokens first
                    head_block(
                        t,
                        (lambda tt: (lambda dc:
                                     xf_sb[:, dc, 128 * tt:128 * (tt + 1)]))(t),
                        t)
                for half in range(2):
                    for s in range(6):
                        pj, tj = s // 2, s % 2
                        head_block(
                            4 + 6 * half + s,
                            (lambda hh, pp_, tt: (lambda dc:
                             xa[:, dc, hh, pp_,
                                128 * tt:128 * (tt + 1)]))(half, pj, tj),
                            s)

    nc.compile()
    return nc


def _make_runner(nc):
    import jax
    import jax.numpy as jnp
    from jax.sharding import Mesh, PartitionSpec, NamedSharding
    from jax.experimental.shard_map import shard_map
    from concourse import bass2jax, mybir

    bass2jax.install_neuronx_cc_hook()
    partition_name = (nc.partition_id_tensor.name
                      if nc.partition_id_tensor else None)

    SHARED = {"te", "wqkv", "wout", "wup", "wdn"}
    in_names, out_names, out_avals = [], [], []
    for alloc in nc.m.functions[0].allocations:
        if not isinstance(alloc, mybir.MemoryLocationSet):
            continue
        name = alloc.memorylocations[0].name
        if alloc.kind == "ExternalInput":
            if name != partition_name:
                in_names.append(name)
        elif alloc.kind == "ExternalOutput":
            out_names.append(name)
            out_avals.append(jax.core.ShapedArray(
                tuple(alloc.tensor_shape), mybir.dt.np(alloc.dtype)))
    n_params = len(in_names)
    full_names = list(in_names) + list(out_names)
    if partition_name is not None:
        full_names.append(partition_name)

    def _body(*args):
        operands = list(args)
        if partition_name is not None:
            operands.append(bass2jax.partition_id_tensor())
        outs = bass2jax._bass_exec_p.bind(
            *operands,
            out_avals=tuple(out_avals),
            in_names=tuple(full_names),
            out_names=tuple(out_names),
            lowering_input_output_aliases=(),
            sim_require_finite=True,
            sim_require_nnan=True,
            nc=nc,
        )
        return tuple(outs)

    devices = jax.devices()[:NCORES]
    mesh = Mesh(np.asarray(devices), ("core",))
    in_specs = tuple(
        PartitionSpec() if n in SHARED else PartitionSpec("core")
        for n in in_names) + (PartitionSpec("core"),) * len(out_names)
    out_specs = (PartitionSpec("core"),) * len(out_names)
    donate = tuple(range(n_params, n_params + len(out_names)))
    sharded = jax.jit(
        shard_map(_body, mesh=mesh, in_specs=in_specs, out_specs=out_specs,
                  check_rep=False),
        donate_argnums=donate, keep_unused=True)

    sharded_nodonate = jax.jit(
        shard_map(_body, mesh=mesh, in_specs=in_specs, out_specs=out_specs,
                  check_rep=False),
        keep_unused=True)

    shard8 = NamedSharding(mesh, PartitionSpec("core"))
    repl = NamedSharding(mesh, PartitionSpec())

    zfns = [
        jax.jit(
            (lambda av: (lambda: jnp.zeros((NCORES * av.shape[0],)
                                           + av.shape[1:], av.dtype)))(av),
            out_shardings=shard8)
        for av in out_avals
    ]

    def put_inputs(per_core_maps, shared_map):
        dev = []
        for n in in_names:
            if n in SHARED:
                dev.append(jax.device_put(shared_map[n], repl))
            else:
                arr = np.concatenate([m[n] for m in per_core_maps], axis=0)
                dev.append(jax.device_put(arr, shard8))
        return dev

    def run(dev_inputs):
        zeros = [zf() for zf in zfns]
        outs = sharded(*dev_inputs, *zeros)
        jax.block_until_ready(outs)
        return {n: outs[i] for i, n in enumerate(out_names)}

    def run_burst(dev_inputs, n):
        zeros = [zf() for zf in zfns]
        jax.block_until_ready(zeros)
        t0 = time.time()
        outs = None
        for _ in range(n):
            outs = sharded_nodonate(*dev_inputs, *zeros)
        jax.block_until_ready(outs)
        return time.time() - t0

    return put_inputs, run, run_burst


def _prepare_inputs(ids, te, pe):
    bf = ml_dtypes.bfloat16
    ids = np.asarray(ids)
    te_f = np.asarray(te, dtype=np.float32)
    per_core = []
    for c in range(NCORES):
        b, cc = c // 4, c % 4
        peers = [r for r in range(4) if r != cc]
        sl = slice(TOK * cc, TOK * (cc + 1))
        idx = ids[b, sl].astype(np.int32).reshape(TOK, 1)
        pe_s = np.asarray(pe[sl], dtype=np.float32)
        # relative diagonal masks: mrel[t][i, j] = 1 if 128*t + i <= j
        ki = (128 * np.arange(4)[:, None, None]
              + np.arange(128)[None, :, None])
        qj = np.arange(TOK)[None, None, :]
        mrel = (ki <= qj).astype(bf)
        # exp bias: own slots 0, peer slot visible iff peer rank < cc
        btab = np.zeros((128, NKT), dtype=np.float32)
        for s in range(12):
            if peers[s // 4] >= cc:
                btab[:, 4 + s] = -30.0
        # gather offsets: row = 768*peer + 128*dc + p
        gofs = np.zeros((128, DC * 3), dtype=np.int32)
        for dc in range(DC):
            for j in range(3):
                gofs[:, 3 * dc + j] = (768 * peers[j] + 128 * dc
                                       + np.arange(128))
        teT_s = np.zeros((D, VPAD), dtype=bf)
        teT_s[:, :VSH] = te_f[VSH * cc:VSH * (cc + 1), :].T.astype(bf)
        if os.environ.get("KERNEL_GOFS0"):
            gofs[:] = 0
        if os.environ.get("KERNEL_BTAB0"):
            btab[:] = 0.0
        if os.environ.get("KERNEL_MREL1"):
            mrel[:] = 1.0
        per_core.append({"idx": idx, "pe_s": pe_s, "mrel": mrel,
                         "btab": btab, "gofs": gofs, "teT_s": teT_s})
    shared_map = {
        "te": te_f,
        "wqkv": _STATE["shared"]["wqkv"],
        "wout": _STATE["shared"]["wout"],
        "wup": _STATE["shared"]["wup"],
        "wdn": _STATE["shared"]["wdn"],
    }
    return per_core, shared_map


def kernel(ids, te, pe, ln1_s, ln1_b, qkv_w, qkv_b, out_w, out_b,
           ln2_s, ln2_b, up_w, up_b, dn_w, dn_b, lnf_s, lnf_b):
    bf = ml_dtypes.bfloat16
    # identity LN affine params and zero biases (true for this model family)
    for z in (ln1_b, ln2_b, lnf_b, qkv_b, out_b, up_b, dn_b):
        assert not np.asarray(z).any(), "nonzero bias unsupported"
    for o in (ln1_s, ln2_s, lnf_s):
        assert np.all(np.asarray(o) == 1.0), "non-identity LN scale unsupported"

    if "run" not in _STATE:
        _STATE["shared"] = {
            "wqkv": np.ascontiguousarray(np.asarray(qkv_w)).astype(bf),
            "wout": np.ascontiguousarray(np.asarray(out_w)).astype(bf),
            "wup": np.ascontiguousarray(np.asarray(up_w)).astype(bf),
            "wdn": np.ascontiguousarray(np.asarray(dn_w)).astype(bf),
        }
        nc = _build_program()
        put_inputs, run, run_burst = _make_runner(nc)
        _STATE["put_inputs"] = put_inputs
        _STATE["run"] = run
        _STATE["run_burst"] = run_burst

    per_core, shared_map = _prepare_inputs(ids, te, pe)
    dev_inputs = _STATE["put_inputs"](per_core, shared_map)
    _STATE["dev_inputs"] = dev_inputs
    outs = _STATE["run"](dev_inputs)
    logits = np.asarray(outs["logits"])  # [8*2048, 8192]
    logits = logits.reshape(NCORES, GTOK, VPAD)[:, :, :VSH]
    # core c = 4b + cc: rows are slot-ordered; map slots back to tokens
    full = np.empty((B, T, V), dtype=np.float32)
    for c in range(NCORES):
        b, cc = c // 4, c % 4
        peers = [r for r in range(4) if r != cc]
        tokmap = np.empty(GTOK, dtype=np.int64)
        for t in range(4):  # own slots
            tokmap[128 * t:128 * (t + 1)] = (TOK * cc + 128 * t
                                             + np.arange(128))
        for half in range(2):
            for s in range(6):
                pj, tj = s // 2, s % 2
                sl_ = 4 + 6 * half + s
                base = TOK * peers[pj] + 256 * half + 128 * tj
                tokmap[128 * sl_:128 * (sl_ + 1)] = base + np.arange(128)
        full[b, tokmap, VSH * cc:VSH * (cc + 1)] = logits[c]
    return full
